# revision 3
# baseline (speedup 1.0000x reference)
"""TRN2 Bass/Tile kernel: 16-head MHA, B=1 S=4096 E=1024, head-sharded over 8 cores.

Sharding: tensor-parallel over heads. Core c owns heads {2c, 2c+1}: columns
[128c, 128(c+1)) of Wq/Wk/Wv (+bias slices) and rows [128c, 128(c+1)) of Wo.
Each core computes attention for its 2 heads and a partial out-projection
[S, E]; the host sums the 8 partials and adds bo (TP row-parallel unshard).

v2 design (PE-queue-bound baseline was 464us: MATMUL 336 + serial LDWEIGHTS 126):
  A) QT/KT/VT [128ch, S] = W_c^T @ x^T   (lhsT=W-slice, rhs=xT tiles, +bias on DVE)
     KT natural (no per-head zero-padding).
  B) V2 [128k, kt, 130] = [V_h0|ones|V_h1|ones] via PE transpose (l-sum ride-along)
  C) per 512-q block, per key-tile kt:
     - QK row-tiled: two concurrent K=64 matmuls (tile_position (0,0)/(64,0))
       -> scores^T [128k, 1024] = [h0 512q | h1 512q] in one PSUM pair
     - exp split by kt parity: even kt on ACT (true exp, scale=1/8), odd kt on
       DVE via Schraudolph fp16-bit trick: int16 = round(a*s + b) bitcast fp16
       (a = 1024*log2e/8, b = 15*1024 + C). Softmax renormalization absorbs
       the approximation's constant factor; mixed-tile error ~5e-3 rel (sim).
     - PV accumulate psum[65, 512] per head: rows 0:64 attn^T, row 64 = denom l
  D) per q-block: recip(l) via DRAM partition-spread, ATT = attn^T * (1/l),
     out-proj [q, E] with PSUM evacuation split ACT/DVE by parity.
"""

import sys

for _p in ("/opt/trn_rl_repo", "/opt/pypackages"):
    if _p not in sys.path:
        sys.path.append(_p)

import numpy as np

EMBED = 1024
N_CORES = 8
HC = EMBED // N_CORES  # 128 channels = 2 heads per core
DH = 64                # head dim
SEQ = 4096

# Schraudolph fp16 exp constants: exp(s/8) ~= bits_fp16(round(EXP_A*s + EXP_B))
EXP_A = 1024.0 * 1.4426950408889634 * 0.125
EXP_B = 15.0 * 1024.0 - 45.0

_NC_CACHE = {}


def _build_nc(S=SEQ, E=EMBED):
    from contextlib import ExitStack

    import concourse.bass as bass
    import concourse.mybir as mybir
    import concourse.tile as tile
    from concourse import bacc
    from concourse.masks import make_identity

    F32 = mybir.dt.float32
    MMDT = mybir.dt.float16
    I16 = mybir.dt.int16

    ET = E // 128      # 8 E-tiles of 128 (contraction for projections)
    NSC = S // 512     # 8 512-wide S chunks
    NKT = S // 128     # 32 128-wide key tiles
    NQS = 512 // 128   # 4 128-q subtiles per chunk
    NEC = E // 512     # 2 512-wide E chunks of the out-projection

    nc = bacc.Bacc()
    xT = nc.declare_dram_parameter("xT", [E, S], MMDT, isOutput=False)
    wq = nc.declare_dram_parameter("wq", [E, HC], MMDT, isOutput=False)
    wk = nc.declare_dram_parameter("wk", [E, HC], MMDT, isOutput=False)
    wv = nc.declare_dram_parameter("wv", [E, HC], MMDT, isOutput=False)
    bq = nc.declare_dram_parameter("bq", [HC, 1], F32, isOutput=False)
    bk = nc.declare_dram_parameter("bk", [HC, 1], F32, isOutput=False)
    bv = nc.declare_dram_parameter("bv", [HC, 1], F32, isOutput=False)
    wo = nc.declare_dram_parameter("wo", [HC, E], MMDT, isOutput=False)
    out = nc.declare_dram_parameter("out", [S, E], F32, isOutput=True)

    with tile.TileContext(nc) as tc, ExitStack() as ctx:
        wpool = ctx.enter_context(tc.tile_pool(name="w", bufs=1))
        xpool = ctx.enter_context(tc.tile_pool(name="x", bufs=4))
        qkvpool = ctx.enter_context(tc.tile_pool(name="qkv", bufs=1))
        v2pool = ctx.enter_context(tc.tile_pool(name="v2", bufs=1))
        epool = ctx.enter_context(tc.tile_pool(name="e", bufs=3))
        apool = ctx.enter_context(tc.tile_pool(name="a", bufs=3))
        rpool = ctx.enter_context(tc.tile_pool(name="r", bufs=4))
        dpool = ctx.enter_context(tc.tile_pool(name="d", bufs=4, space="DRAM"))
        # PSUM 8 banks: scores double-buffer 2x[128,1024]=4, PV 2x[65,512]=2,
        # transpose/out-proj 2x[128,512]=2
        spsum = ctx.enter_context(tc.tile_pool(name="sp", bufs=2, space="PSUM"))
        pvpsum = ctx.enter_context(tc.tile_pool(name="pv", bufs=2, space="PSUM"))
        opsum = ctx.enter_context(tc.tile_pool(name="op", bufs=2, space="PSUM"))

        # --- weights / constants ---
        w_sb = {}
        for name, src in (("wq", wq), ("wk", wk), ("wv", wv)):
            t = wpool.tile([128, ET, HC], MMDT, tag=name, name=name)
            nc.sync.dma_start(out=t, in_=src.rearrange("(a p) c -> p a c", p=128))
            w_sb[name] = t
        wo_sb = wpool.tile([HC, E], MMDT, tag="wo")
        nc.sync.dma_start(out=wo_sb, in_=wo[:, :])
        b_sb = {}
        for name, src in (("bq", bq), ("bk", bk), ("bv", bv)):
            t = wpool.tile([HC, 1], F32, tag=name, name=name)
            nc.sync.dma_start(out=t, in_=src[:, :])
            b_sb[name] = t
        ident = wpool.tile([128, 128], MMDT, tag="ident")
        make_identity(nc, ident)

        # --- stage A: QT/KT/VT [128ch, S] chunked by 512, natural head layout ---
        QT = [qkvpool.tile([HC, 512], MMDT, tag=f"qt{i}", name=f"qt{i}") for i in range(NSC)]
        KT = [qkvpool.tile([HC, 512], MMDT, tag=f"kt{i}", name=f"kt{i}") for i in range(NSC)]
        VT = [qkvpool.tile([HC, 512], MMDT, tag=f"vt{i}", name=f"vt{i}") for i in range(NSC)]
        for sc in range(NSC):
            big1 = spsum.tile([128, 1024], F32, tag="big")
            big2 = opsum.tile([128, 512], F32, tag="pt_po")
            for et in range(ET):
                xt = xpool.tile([128, 512], MMDT, tag="xt")
                nc.sync.dma_start(
                    out=xt, in_=xT[et * 128:(et + 1) * 128, sc * 512:(sc + 1) * 512]
                )
                first, last = et == 0, et == ET - 1
                nc.tensor.matmul(big1[:, 0:512], lhsT=w_sb["wq"][:, et, :],
                                 rhs=xt, start=first, stop=last)
                nc.tensor.matmul(big1[:, 512:1024], lhsT=w_sb["wk"][:, et, :],
                                 rhs=xt, start=first, stop=last)
                nc.tensor.matmul(big2[:, 0:512], lhsT=w_sb["wv"][:, et, :],
                                 rhs=xt, start=first, stop=last)
            nc.vector.tensor_scalar_add(QT[sc], big1[:, 0:512], b_sb["bq"])
            nc.vector.tensor_scalar_add(KT[sc], big1[:, 512:1024], b_sb["bk"])
            nc.vector.tensor_scalar_add(VT[sc], big2[:, 0:512], b_sb["bv"])

        # --- stage B: V2 [128k, NKT, 130] = [V_h0|ones|V_h1|ones] ---
        V2 = v2pool.tile([128, NKT, 130], MMDT, tag="V2")
        nc.vector.memset(V2[:, :, 64:65], 1.0)
        nc.vector.memset(V2[:, :, 129:130], 1.0)
        for kt in range(NKT):
            pt = opsum.tile([128, 512], MMDT, tag="pt_po")
            nc.tensor.transpose(
                pt[:, 0:128], VT[kt // 4][:, (kt % 4) * 128:(kt % 4 + 1) * 128], ident
            )
            # one copy: [128, 2, 64] -> V2 cols {0:64, 65:129}
            dst = bass.AP(
                tensor=V2.tensor, offset=V2.offset + kt * 130,
                ap=[list(V2.ap[0]), [65, 2], [1, 64]],
            )
            src = bass.AP(
                tensor=pt.tensor, offset=pt.offset,
                ap=[list(pt.ap[0]), [64, 2], [1, 64]],
            )
            nc.vector.tensor_copy(dst, src)

        # --- stage C+D: per 512-q block over all 32 key tiles, both heads ---
        for qb in range(NSC):
            pv0 = pvpsum.tile([65, 512], F32, tag="pv", name="pv0")
            pv1 = pvpsum.tile([65, 512], F32, tag="pv", name="pv1")
            for kt in range(NKT):
                s = spsum.tile([128, 1024], F32, tag="big")
                ktile = KT[kt // 4][:, (kt % 4) * 128:(kt % 4 + 1) * 128]
                nc.tensor.matmul(
                    s[:, 0:512], lhsT=ktile[0:DH, :], rhs=QT[qb][0:DH, :],
                    start=True, stop=True,
                )
                nc.tensor.matmul(
                    s[:, 512:1024], lhsT=ktile[DH:HC, :], rhs=QT[qb][DH:HC, :],
                    start=True, stop=True,
                )
                ex = epool.tile([128, 1024], MMDT, tag="ex")
                if kt % 2 == 0:
                    nc.scalar.activation(
                        ex, s, mybir.ActivationFunctionType.Exp, scale=0.125,
                    )
                else:
                    nc.vector.tensor_scalar(
                        out=ex.bitcast(I16), in0=s,
                        scalar1=EXP_A, scalar2=EXP_B,
                        op0=mybir.AluOpType.mult, op1=mybir.AluOpType.add,
                    )
                first, last = kt == 0, kt == NKT - 1
                nc.tensor.matmul(
                    pv0, lhsT=V2[:, kt, 0:65], rhs=ex[:, 0:512],
                    start=first, stop=last,
                )
                nc.tensor.matmul(
                    pv1, lhsT=V2[:, kt, 65:130], rhs=ex[:, 512:1024],
                    start=first, stop=last,
                )
            # --- stage D: normalize, project, store ---
            ATT = apool.tile([128, 512], MMDT, tag="att")
            for h, pv in ((0, pv0), (1, pv1)):
                hs = slice(h * DH, (h + 1) * DH)
                pvc = rpool.tile([65, 512], F32, tag=f"pvc{h}", name="pvc")
                nc.vector.tensor_copy(pvc, pv)
                # reciprocal of the l-row, partition-spread for lane
                # parallelism: [1,512] -> dram -> [128,4] -> recip -> back
                scr = dpool.tile([1, 512], F32, tag="scr")
                nc.sync.dma_start(out=scr, in_=pvc[64:65, :])
                rsp = rpool.tile([128, 4], F32, tag="rsp")
                nc.sync.dma_start(
                    out=rsp,
                    in_=bass.AP(tensor=scr.tensor, offset=scr.offset,
                                ap=[[1, 128], [128, 4]]),
                )
                rsp2 = rpool.tile([128, 4], F32, tag="rsp2")
                nc.vector.reciprocal(rsp2, rsp)
                scr2 = dpool.tile([1, 512], F32, tag="scr2")
                nc.sync.dma_start(
                    out=bass.AP(tensor=scr2.tensor, offset=scr2.offset,
                                ap=[[1, 128], [128, 4]]),
                    in_=rsp2,
                )
                bc = rpool.tile([DH, 512], F32, tag="bc")
                nc.sync.dma_start(
                    out=bc,
                    in_=bass.AP(tensor=scr2.tensor, offset=scr2.offset,
                                ap=[[0, DH]] + list(scr2.ap)[1:]),
                )
                nc.vector.tensor_mul(ATT[hs, :], pvc[0:DH, :], bc)
            for qs in range(NQS):
                for ec in range(NEC):
                    po = opsum.tile([128, 512], F32, tag="pt_po")
                    nc.tensor.matmul(
                        po,
                        lhsT=ATT[:, qs * 128:(qs + 1) * 128],
                        rhs=wo_sb[:, ec * 512:(ec + 1) * 512],
                        start=True, stop=True,
                    )
                    osb = apool.tile([128, 512], F32, tag="osb")
                    if (qs * NEC + ec) % 2 == 0:
                        nc.vector.tensor_copy(osb, po)
                    else:
                        nc.scalar.copy(osb, po)
                    nc.sync.dma_start(
                        out=out[qb * 512 + qs * 128:qb * 512 + (qs + 1) * 128,
                                ec * 512:(ec + 1) * 512],
                        in_=osb,
                    )
    nc.finalize()
    return nc


def _get_nc(S=SEQ):
    key = S
    if key not in _NC_CACHE:
        _NC_CACHE[key] = _build_nc(S=S)
    return _NC_CACHE[key]


def _make_in_maps(x, Wq, bq, Wk, bk, Wv, bv, Wo, npdt=np.float16):
    xT = np.ascontiguousarray(np.asarray(x, np.float32)[0].T.astype(npdt))
    Wq, Wk, Wv, Wo = (np.asarray(a, np.float32).astype(npdt) for a in (Wq, Wk, Wv, Wo))
    bq, bk, bv = (np.asarray(a, np.float32) for a in (bq, bk, bv))
    in_maps = []
    for c in range(N_CORES):
        sl = slice(c * HC, (c + 1) * HC)
        in_maps.append({
            "xT": xT,
            "wq": np.ascontiguousarray(Wq[:, sl]),
            "wk": np.ascontiguousarray(Wk[:, sl]),
            "wv": np.ascontiguousarray(Wv[:, sl]),
            "bq": np.ascontiguousarray(bq[sl]).reshape(HC, 1),
            "bk": np.ascontiguousarray(bk[sl]).reshape(HC, 1),
            "bv": np.ascontiguousarray(bv[sl]).reshape(HC, 1),
            "wo": np.ascontiguousarray(Wo[sl, :]),
        })
    return in_maps


def run(inputs, trace=False, mmdt="fp16"):
    """Run the kernel; returns (out [1,S,E] float32, BassKernelResults)."""
    from concourse.bass_utils import run_bass_kernel_spmd

    nc = _get_nc()
    in_maps = _make_in_maps(
        inputs["x"], inputs["Wq"], inputs["bq"], inputs["Wk"], inputs["bk"],
        inputs["Wv"], inputs["bv"], inputs["Wo"],
    )
    res = run_bass_kernel_spmd(
        nc, in_maps, core_ids=list(range(N_CORES)), trace=trace
    )
    acc = np.zeros((SEQ, EMBED), np.float64)
    for c in range(N_CORES):
        acc += res.results[c]["out"]
    acc += np.asarray(inputs["bo"], np.float64)
    return acc.astype(np.float32).reshape(1, SEQ, EMBED), res


def kernel(x, Wq, bq, Wk, bk, Wv, bv, Wo, bo):
    out, _ = run(dict(x=x, Wq=Wq, bq=bq, Wk=Wk, bk=bk, Wv=Wv, bv=bv, Wo=Wo, bo=bo))
    return out


# revision 6
# speedup vs baseline: 1.1534x; 1.1534x over previous
"""TRN2 Bass/Tile kernel: 16-head MHA, B=1 S=4096 E=1024, head-sharded over 8 cores.

Sharding: tensor-parallel over heads. Core c owns heads {2c, 2c+1}: columns
[128c, 128(c+1)) of Wq/Wk/Wv (+bias slices) and rows [128c, 128(c+1)) of Wo.
Each core computes attention for its 2 heads and a partial out-projection
[S, E]; the host sums the 8 partials and adds bo (TP row-parallel unshard).

v2 design (PE-queue-bound baseline was 464us: MATMUL 336 + serial LDWEIGHTS 126):
  A) QT/KT/VT [128ch, S] = W_c^T @ x^T   (lhsT=W-slice, rhs=xT tiles, +bias on DVE)
     KT natural (no per-head zero-padding).
  B) V2 [128k, kt, 130] = [V_h0|ones|V_h1|ones] via PE transpose (l-sum ride-along)
  C) per 512-q block, per key-tile kt:
     - QK row-tiled: two concurrent K=64 matmuls (tile_position (0,0)/(64,0))
       -> scores^T [128k, 1024] = [h0 512q | h1 512q] in one PSUM pair
     - exp split by kt parity: even kt on ACT (true exp, scale=1/8), odd kt on
       DVE via Schraudolph fp16-bit trick: int16 = round(a*s + b) bitcast fp16
       (a = 1024*log2e/8, b = 15*1024 + C). Softmax renormalization absorbs
       the approximation's constant factor; mixed-tile error ~5e-3 rel (sim).
     - PV accumulate psum[65, 512] per head: rows 0:64 attn^T, row 64 = denom l
  D) per q-block: recip(l) via DRAM partition-spread, ATT = attn^T * (1/l),
     out-proj [q, E] with PSUM evacuation split ACT/DVE by parity.
"""

import sys

for _p in ("/opt/trn_rl_repo", "/opt/pypackages"):
    if _p not in sys.path:
        sys.path.append(_p)

import numpy as np

EMBED = 1024
N_CORES = 8
HC = EMBED // N_CORES  # 128 channels = 2 heads per core
DH = 64                # head dim
SEQ = 4096

# Schraudolph fp16 exp constants: exp(s/8) ~= bits_fp16(round(EXP_A*s + EXP_B))
EXP_A = 1024.0 * 1.4426950408889634 * 0.125
EXP_B = 15.0 * 1024.0 - 45.0

_NC_CACHE = {}


def _build_nc(S=SEQ, E=EMBED):
    from contextlib import ExitStack

    import concourse.bass as bass
    import concourse.mybir as mybir
    import concourse.tile as tile
    from concourse import bacc
    from concourse.masks import make_identity

    F32 = mybir.dt.float32
    MMDT = mybir.dt.float16
    I16 = mybir.dt.int16

    ET = E // 128      # 8 E-tiles of 128 (contraction for projections)
    NSC = S // 512     # 8 512-wide S chunks
    NKT = S // 128     # 32 128-wide key tiles
    NQS = 512 // 128   # 4 128-q subtiles per chunk
    NEC = E // 512     # 2 512-wide E chunks of the out-projection

    nc = bacc.Bacc()
    xT = nc.declare_dram_parameter("xT", [E, S], MMDT, isOutput=False)
    wq = nc.declare_dram_parameter("wq", [E, HC], MMDT, isOutput=False)
    wk = nc.declare_dram_parameter("wk", [E, HC], MMDT, isOutput=False)
    wv = nc.declare_dram_parameter("wv", [E, HC], MMDT, isOutput=False)
    bq = nc.declare_dram_parameter("bq", [HC, 1], F32, isOutput=False)
    bk = nc.declare_dram_parameter("bk", [HC, 1], F32, isOutput=False)
    bv = nc.declare_dram_parameter("bv", [HC, 1], F32, isOutput=False)
    wo = nc.declare_dram_parameter("wo", [HC, E], MMDT, isOutput=False)
    out = nc.declare_dram_parameter("out", [S, E], F32, isOutput=True)

    with tile.TileContext(nc) as tc, ExitStack() as ctx:
        wpool = ctx.enter_context(tc.tile_pool(name="w", bufs=1))
        xpool = ctx.enter_context(tc.tile_pool(name="x", bufs=8))
        qkvpool = ctx.enter_context(tc.tile_pool(name="qkv", bufs=1))
        v2pool = ctx.enter_context(tc.tile_pool(name="v2", bufs=1))
        epool = ctx.enter_context(tc.tile_pool(name="e", bufs=4))
        apool = ctx.enter_context(tc.tile_pool(name="a", bufs=3))
        rpool = ctx.enter_context(tc.tile_pool(name="r", bufs=4))
        dpool = ctx.enter_context(tc.tile_pool(name="d", bufs=4, space="DRAM"))
        # PSUM 8 banks: scores double-buffer 2x[128,1024]=4, PV 2x[65,512]=2,
        # transpose/out-proj 2x[128,512]=2
        spsum = ctx.enter_context(tc.tile_pool(name="sp", bufs=2, space="PSUM"))
        pvpsum = ctx.enter_context(tc.tile_pool(name="pv", bufs=2, space="PSUM"))
        opsum = ctx.enter_context(tc.tile_pool(name="op", bufs=2, space="PSUM"))

        # --- weights / constants ---
        w_sb = {}
        for name, src in (("wq", wq), ("wk", wk), ("wv", wv)):
            t = wpool.tile([128, ET, HC], MMDT, tag=name, name=name)
            nc.sync.dma_start(out=t, in_=src.rearrange("(a p) c -> p a c", p=128))
            w_sb[name] = t
        wo_sb = wpool.tile([HC, E], MMDT, tag="wo")
        nc.sync.dma_start(out=wo_sb, in_=wo[:, :])
        b_sb = {}
        for name, src in (("bq", bq), ("bk", bk), ("bv", bv)):
            t = wpool.tile([HC, 1], F32, tag=name, name=name)
            nc.sync.dma_start(out=t, in_=src[:, :])
            b_sb[name] = t
        ident = wpool.tile([128, 128], MMDT, tag="ident")
        make_identity(nc, ident)

        # --- stage A: QT/KT/VT [128ch, S] chunked by 512, natural head layout ---
        QT = [qkvpool.tile([HC, 512], MMDT, tag=f"qt{i}", name=f"qt{i}") for i in range(NSC)]
        KT = [qkvpool.tile([HC, 512], MMDT, tag=f"kt{i}", name=f"kt{i}") for i in range(NSC)]
        VT = [qkvpool.tile([HC, 512], MMDT, tag=f"vt{i}", name=f"vt{i}") for i in range(NSC)]
        for sc in range(NSC):
            big1 = spsum.tile([128, 1024], F32, tag="big")
            big2 = opsum.tile([128, 512], F32, tag="pt_po")
            for et in range(ET):
                xt = xpool.tile([128, 512], MMDT, tag="xt")
                nc.sync.dma_start(
                    out=xt, in_=xT[et * 128:(et + 1) * 128, sc * 512:(sc + 1) * 512]
                )
                first, last = et == 0, et == ET - 1
                nc.tensor.matmul(big1[:, 0:512], lhsT=w_sb["wq"][:, et, :],
                                 rhs=xt, start=first, stop=last)
                nc.tensor.matmul(big1[:, 512:1024], lhsT=w_sb["wk"][:, et, :],
                                 rhs=xt, start=first, stop=last)
                nc.tensor.matmul(big2[:, 0:512], lhsT=w_sb["wv"][:, et, :],
                                 rhs=xt, start=first, stop=last)
            nc.vector.tensor_scalar_add(QT[sc], big1[:, 0:512], b_sb["bq"])
            nc.vector.tensor_scalar_add(KT[sc], big1[:, 512:1024], b_sb["bk"])
            nc.vector.tensor_scalar_add(VT[sc], big2[:, 0:512], b_sb["bv"])

        # --- stage B: V2 [128k, NKT, 130] = [V_h0|ones|V_h1|ones] ---
        V2 = v2pool.tile([128, NKT, 130], MMDT, tag="V2")
        nc.vector.memset(V2[:, :, 64:65], 1.0)
        nc.vector.memset(V2[:, :, 129:130], 1.0)
        for kt in range(NKT):
            pt = opsum.tile([128, 512], MMDT, tag="pt_po")
            nc.tensor.transpose(
                pt[:, 0:128], VT[kt // 4][:, (kt % 4) * 128:(kt % 4 + 1) * 128], ident
            )
            # one copy: [128, 2, 64] -> V2 cols {0:64, 65:129}
            dst = bass.AP(
                tensor=V2.tensor, offset=V2.offset + kt * 130,
                ap=[list(V2.ap[0]), [65, 2], [1, 64]],
            )
            src = bass.AP(
                tensor=pt.tensor, offset=pt.offset,
                ap=[list(pt.ap[0]), [64, 2], [1, 64]],
            )
            nc.vector.tensor_copy(dst, src)

        # --- stage C+D: per 512-q block over all 32 key tiles, both heads.
        # Stage D is split into three phases, each deferred one q-block
        # behind, so neither the strict-FIFO ACT/DVE queues nor the PE queue
        # ever wait on the reciprocal's DRAM round-trips:
        #   phase1(qb): kt loop + PV-psum evacuation + l-row spread DMAs
        #   phase2(qb): reciprocal + spread-back DMAs     (at end of qb+1)
        #   phase3(qb): ATT scale + out-proj + store      (at end of qb+2)
        def phase1(qb):
            pv0 = pvpsum.tile([65, 512], F32, tag="pv", name="pv0")
            pv1 = pvpsum.tile([65, 512], F32, tag="pv", name="pv1")
            for kt in range(NKT):
                s = spsum.tile([128, 1024], F32, tag="big")
                ktile = KT[kt // 4][:, (kt % 4) * 128:(kt % 4 + 1) * 128]
                nc.tensor.matmul(
                    s[:, 0:512], lhsT=ktile[0:DH, :], rhs=QT[qb][0:DH, :],
                    start=True, stop=True,
                )
                nc.tensor.matmul(
                    s[:, 512:1024], lhsT=ktile[DH:HC, :], rhs=QT[qb][DH:HC, :],
                    start=True, stop=True,
                )
                ex = epool.tile([128, 1024], MMDT, tag="ex")
                if kt % 2 == 0:
                    nc.scalar.activation(
                        ex, s, mybir.ActivationFunctionType.Exp, scale=0.125,
                    )
                else:
                    nc.vector.tensor_scalar(
                        out=ex.bitcast(I16), in0=s,
                        scalar1=EXP_A, scalar2=EXP_B,
                        op0=mybir.AluOpType.mult, op1=mybir.AluOpType.add,
                    )
                first, last = kt == 0, kt == NKT - 1
                nc.tensor.matmul(
                    pv0, lhsT=V2[:, kt, 0:65], rhs=ex[:, 0:512],
                    start=first, stop=last,
                )
                nc.tensor.matmul(
                    pv1, lhsT=V2[:, kt, 65:130], rhs=ex[:, 512:1024],
                    start=first, stop=last,
                )
            st = {"pvcs": [], "rsps": [], "qb": qb}
            for h, pv in ((0, pv0), (1, pv1)):
                pvc = rpool.tile([65, 512], F32, tag=f"pvc{h}", name="pvc")
                nc.vector.tensor_copy(pvc, pv)
                # l-row [1,512] -> dram -> [128,4] partition spread
                scr = dpool.tile([1, 512], F32, tag=f"scr{h}")
                nc.sync.dma_start(out=scr, in_=pvc[64:65, :])
                rsp = rpool.tile([128, 4], F32, tag=f"rsp{h}")
                nc.sync.dma_start(
                    out=rsp,
                    in_=bass.AP(tensor=scr.tensor, offset=scr.offset,
                                ap=[[1, 128], [128, 4]]),
                )
                st["pvcs"].append(pvc)
                st["rsps"].append(rsp)
            return st

        def phase2(st):
            st["bcs"] = []
            for h in range(2):
                rsp2 = rpool.tile([128, 4], F32, tag=f"rsp2{h}")
                nc.vector.reciprocal(rsp2, st["rsps"][h])
                scr2 = dpool.tile([1, 512], F32, tag=f"scr2{h}")
                nc.sync.dma_start(
                    out=bass.AP(tensor=scr2.tensor, offset=scr2.offset,
                                ap=[[1, 128], [128, 4]]),
                    in_=rsp2,
                )
                bc = rpool.tile([DH, 512], F32, tag=f"bc{h}")
                nc.sync.dma_start(
                    out=bc,
                    in_=bass.AP(tensor=scr2.tensor, offset=scr2.offset,
                                ap=[[0, DH]] + list(scr2.ap)[1:]),
                )
                st["bcs"].append(bc)

        def phase3(st):
            qb = st["qb"]
            ATT = apool.tile([128, 512], MMDT, tag="att")
            for h in range(2):
                hs = slice(h * DH, (h + 1) * DH)
                nc.vector.tensor_mul(ATT[hs, :], st["pvcs"][h][0:DH, :],
                                     st["bcs"][h])
            for qs in range(NQS):
                for ec in range(NEC):
                    po = opsum.tile([128, 512], F32, tag="pt_po")
                    nc.tensor.matmul(
                        po,
                        lhsT=ATT[:, qs * 128:(qs + 1) * 128],
                        rhs=wo_sb[:, ec * 512:(ec + 1) * 512],
                        start=True, stop=True,
                    )
                    osb = apool.tile([128, 512], F32, tag="osb")
                    if (qs * NEC + ec) % 2 == 0:
                        nc.vector.tensor_copy(osb, po)
                    else:
                        nc.scalar.copy(osb, po)
                    nc.sync.dma_start(
                        out=out[qb * 512 + qs * 128:qb * 512 + (qs + 1) * 128,
                                ec * 512:(ec + 1) * 512],
                        in_=osb,
                    )

        pending = []
        for qb in range(NSC):
            st = phase1(qb)
            pending.append(st)
            if len(pending) >= 2:
                phase2(pending[-2])
            if len(pending) >= 3:
                phase3(pending.pop(0))
        phase2(pending[-1])
        phase3(pending.pop(0))
        phase3(pending.pop(0))
    nc.finalize()
    return nc


def _get_nc(S=SEQ):
    key = S
    if key not in _NC_CACHE:
        _NC_CACHE[key] = _build_nc(S=S)
    return _NC_CACHE[key]


def _make_in_maps(x, Wq, bq, Wk, bk, Wv, bv, Wo, npdt=np.float16):
    xT = np.ascontiguousarray(np.asarray(x, np.float32)[0].T.astype(npdt))
    Wq, Wk, Wv, Wo = (np.asarray(a, np.float32).astype(npdt) for a in (Wq, Wk, Wv, Wo))
    bq, bk, bv = (np.asarray(a, np.float32) for a in (bq, bk, bv))
    in_maps = []
    for c in range(N_CORES):
        sl = slice(c * HC, (c + 1) * HC)
        in_maps.append({
            "xT": xT,
            "wq": np.ascontiguousarray(Wq[:, sl]),
            "wk": np.ascontiguousarray(Wk[:, sl]),
            "wv": np.ascontiguousarray(Wv[:, sl]),
            "bq": np.ascontiguousarray(bq[sl]).reshape(HC, 1),
            "bk": np.ascontiguousarray(bk[sl]).reshape(HC, 1),
            "bv": np.ascontiguousarray(bv[sl]).reshape(HC, 1),
            "wo": np.ascontiguousarray(Wo[sl, :]),
        })
    return in_maps


def run(inputs, trace=False, mmdt="fp16"):
    """Run the kernel; returns (out [1,S,E] float32, BassKernelResults)."""
    from concourse.bass_utils import run_bass_kernel_spmd

    nc = _get_nc()
    in_maps = _make_in_maps(
        inputs["x"], inputs["Wq"], inputs["bq"], inputs["Wk"], inputs["bk"],
        inputs["Wv"], inputs["bv"], inputs["Wo"],
    )
    res = run_bass_kernel_spmd(
        nc, in_maps, core_ids=list(range(N_CORES)), trace=trace
    )
    acc = np.zeros((SEQ, EMBED), np.float64)
    for c in range(N_CORES):
        acc += res.results[c]["out"]
    acc += np.asarray(inputs["bo"], np.float64)
    return acc.astype(np.float32).reshape(1, SEQ, EMBED), res


def kernel(x, Wq, bq, Wk, bk, Wv, bv, Wo, bo):
    out, _ = run(dict(x=x, Wq=Wq, bq=bq, Wk=Wk, bk=bk, Wv=Wv, bv=bv, Wo=Wo, bo=bo))
    return out


# revision 10
# speedup vs baseline: 1.3867x; 1.2023x over previous
"""TRN2 Bass/Tile kernel: 16-head MHA, B=1 S=4096 E=1024, head-sharded over 8 cores.

Sharding: tensor-parallel over heads. Core c owns heads {2c, 2c+1}: columns
[128c, 128(c+1)) of Wq/Wk/Wv (+bias slices) and rows [128c, 128(c+1)) of Wo.
Each core computes attention for its 2 heads and a partial out-projection
[S, E]; the host sums the 8 partials and adds bo (TP row-parallel unshard).

v2 design (PE-queue-bound baseline was 464us: MATMUL 336 + serial LDWEIGHTS 126):
  A) QT/KT/VT [128ch, S] = W_c^T @ x^T   (lhsT=W-slice, rhs=xT tiles, +bias on DVE)
     KT natural (no per-head zero-padding).
  B) V2 [128k, kt, 130] = [V_h0|ones|V_h1|ones] via PE transpose (l-sum ride-along)
  C) per 512-q block, per key-tile kt:
     - QK row-tiled: two concurrent K=64 matmuls (tile_position (0,0)/(64,0))
       -> scores^T [128k, 1024] = [h0 512q | h1 512q] in one PSUM pair
     - exp split by kt parity: even kt on ACT (true exp, scale=1/8), odd kt on
       DVE via Schraudolph fp16-bit trick: int16 = round(a*s + b) bitcast fp16
       (a = 1024*log2e/8, b = 15*1024 + C). Softmax renormalization absorbs
       the approximation's constant factor; mixed-tile error ~5e-3 rel (sim).
     - PV accumulate psum[65, 512] per head: rows 0:64 attn^T, row 64 = denom l
  D) per q-block: recip(l) via DRAM partition-spread, ATT = attn^T * (1/l),
     out-proj [q, E] with PSUM evacuation split ACT/DVE by parity.
"""

import sys

for _p in ("/opt/trn_rl_repo", "/opt/pypackages"):
    if _p not in sys.path:
        sys.path.append(_p)

import numpy as np

EMBED = 1024
N_CORES = 8
HC = EMBED // N_CORES  # 128 channels = 2 heads per core
DH = 64                # head dim
SEQ = 4096

# Schraudolph fp16 exp constants: exp(s/8) ~= bits_fp16(round(EXP_A*s + EXP_B))
EXP_A = 1024.0 * 1.4426950408889634 * 0.125
EXP_B = 15.0 * 1024.0 - 45.0

_NC_CACHE = {}


def _build_nc(S=SEQ, E=EMBED):
    from contextlib import ExitStack

    import concourse.bass as bass
    import concourse.mybir as mybir
    import concourse.tile as tile
    from concourse import bacc
    from concourse.masks import make_identity

    F32 = mybir.dt.float32
    MMDT = mybir.dt.float16
    I16 = mybir.dt.int16

    ET = E // 128      # 8 E-tiles of 128 (contraction for projections)
    NSC = S // 512     # 8 512-wide S chunks
    NKT = S // 128     # 32 128-wide key tiles
    NQS = 512 // 128   # 4 128-q subtiles per chunk
    NEC = E // 512     # 2 512-wide E chunks of the out-projection

    nc = bacc.Bacc()
    xT = nc.declare_dram_parameter("xT", [E, S], MMDT, isOutput=False)
    wq = nc.declare_dram_parameter("wq", [E, HC], MMDT, isOutput=False)
    wk = nc.declare_dram_parameter("wk", [E, HC], MMDT, isOutput=False)
    wv = nc.declare_dram_parameter("wv", [E, HC], MMDT, isOutput=False)
    bq = nc.declare_dram_parameter("bq", [HC, 1], F32, isOutput=False)
    bk = nc.declare_dram_parameter("bk", [HC, 1], F32, isOutput=False)
    bv = nc.declare_dram_parameter("bv", [HC, 1], F32, isOutput=False)
    wo = nc.declare_dram_parameter("wo", [HC, E], MMDT, isOutput=False)
    out = nc.declare_dram_parameter("out", [S, E], F32, isOutput=True)

    with tile.TileContext(nc) as tc, ExitStack() as ctx:
        wpool = ctx.enter_context(tc.tile_pool(name="w", bufs=1))
        xpool = ctx.enter_context(tc.tile_pool(name="x", bufs=8))
        qkvpool = ctx.enter_context(tc.tile_pool(name="qkv", bufs=1))
        v2pool = ctx.enter_context(tc.tile_pool(name="v2", bufs=1))
        epool = ctx.enter_context(tc.tile_pool(name="e", bufs=6))
        apool = ctx.enter_context(tc.tile_pool(name="a", bufs=3))
        rpool = ctx.enter_context(tc.tile_pool(name="r", bufs=4))
        dpool = ctx.enter_context(tc.tile_pool(name="d", bufs=4, space="DRAM"))
        # PSUM 8 banks: scores double-buffer 2x[128,1024]=4, PV 2x[65,512]=2,
        # transpose/out-proj 2x[128,512]=2
        spsum = ctx.enter_context(tc.tile_pool(name="sp", bufs=2, space="PSUM"))
        pvpsum = ctx.enter_context(tc.tile_pool(name="pv", bufs=2, space="PSUM"))
        opsum = ctx.enter_context(tc.tile_pool(name="op", bufs=2, space="PSUM"))

        # --- weights / constants ---
        w_sb = {}
        for name, src in (("wq", wq), ("wk", wk), ("wv", wv)):
            t = wpool.tile([128, ET, HC], MMDT, tag=name, name=name)
            nc.sync.dma_start(out=t, in_=src.rearrange("(a p) c -> p a c", p=128))
            w_sb[name] = t
        wo_sb = wpool.tile([HC, E], MMDT, tag="wo")
        nc.sync.dma_start(out=wo_sb, in_=wo[:, :])
        b_sb = {}
        for name, src in (("bq", bq), ("bk", bk), ("bv", bv)):
            t = wpool.tile([HC, 1], F32, tag=name, name=name)
            nc.sync.dma_start(out=t, in_=src[:, :])
            b_sb[name] = t
        ident = wpool.tile([128, 128], MMDT, tag="ident")
        make_identity(nc, ident)

        # --- stage A: QT/KT/VT [128ch, S] chunked by 512, natural head layout ---
        QT = [qkvpool.tile([HC, 512], MMDT, tag=f"qt{i}", name=f"qt{i}") for i in range(NSC)]
        KT = [qkvpool.tile([HC, 512], MMDT, tag=f"kt{i}", name=f"kt{i}") for i in range(NSC)]
        VT = [qkvpool.tile([HC, 512], MMDT, tag=f"vt{i}", name=f"vt{i}") for i in range(NSC)]
        for sc in range(NSC):
            big1 = spsum.tile([128, 1024], F32, tag="big")
            big2 = opsum.tile([128, 512], F32, tag="pt_po")
            for et in range(ET):
                xt = xpool.tile([128, 512], MMDT, tag="xt")
                nc.sync.dma_start(
                    out=xt, in_=xT[et * 128:(et + 1) * 128, sc * 512:(sc + 1) * 512]
                )
                first, last = et == 0, et == ET - 1
                nc.tensor.matmul(big1[:, 0:512], lhsT=w_sb["wq"][:, et, :],
                                 rhs=xt, start=first, stop=last)
                nc.tensor.matmul(big1[:, 512:1024], lhsT=w_sb["wk"][:, et, :],
                                 rhs=xt, start=first, stop=last)
                nc.tensor.matmul(big2[:, 0:512], lhsT=w_sb["wv"][:, et, :],
                                 rhs=xt, start=first, stop=last)
            nc.vector.tensor_scalar_add(QT[sc], big1[:, 0:512], b_sb["bq"])
            nc.vector.tensor_scalar_add(KT[sc], big1[:, 512:1024], b_sb["bk"])
            nc.vector.tensor_scalar_add(VT[sc], big2[:, 0:512], b_sb["bv"])

        # --- stage B: V2 [128k, NKT, 130] = [V_h0|ones|V_h1|ones] ---
        V2 = v2pool.tile([128, NKT, 130], MMDT, tag="V2")
        nc.vector.memset(V2[:, :, 64:65], 1.0)
        nc.vector.memset(V2[:, :, 129:130], 1.0)
        for kt in range(NKT):
            pt = opsum.tile([128, 512], MMDT, tag="pt_po")
            nc.tensor.transpose(
                pt[:, 0:128], VT[kt // 4][:, (kt % 4) * 128:(kt % 4 + 1) * 128], ident
            )
            # one copy: [128, 2, 64] -> V2 cols {0:64, 65:129}
            dst = bass.AP(
                tensor=V2.tensor, offset=V2.offset + kt * 130,
                ap=[list(V2.ap[0]), [65, 2], [1, 64]],
            )
            src = bass.AP(
                tensor=pt.tensor, offset=pt.offset,
                ap=[list(pt.ap[0]), [64, 2], [1, 64]],
            )
            nc.vector.tensor_copy(dst, src)

        # --- stage C+D: per 512-q block over all 32 key tiles, both heads.
        # Stage D is split into three phases, each deferred one q-block
        # behind, so neither the strict-FIFO ACT/DVE queues nor the PE queue
        # ever wait on the reciprocal's DRAM round-trips:
        #   phase1(qb): kt loop + PV-psum evacuation + l-row spread DMAs
        #   phase2(qb): reciprocal + spread-back DMAs     (at end of qb+1)
        #   phase3(qb): ATT scale + out-proj + store      (at end of qb+2)
        def phase1(qb, tail3):
            """kt loop, software-pipelined emission: QK(kt) | exp(kt-1) |
            PV(kt-2), with the previous-previous block's phase3 work (tail3)
            spread through the early iterations as PE/ACT filler."""
            pv0 = pvpsum.tile([65, 512], F32, tag="pv", name="pv0")
            pv1 = pvpsum.tile([65, 512], F32, tag="pv", name="pv1")
            stiles = {}
            extiles = {}

            def emit_qk(kt):
                s = spsum.tile([128, 1024], F32, tag="big")
                ktile = KT[kt // 4][:, (kt % 4) * 128:(kt % 4 + 1) * 128]
                nc.tensor.matmul(
                    s[:, 0:512], lhsT=ktile[0:DH, :], rhs=QT[qb][0:DH, :],
                    start=True, stop=True,
                )
                nc.tensor.matmul(
                    s[:, 512:1024], lhsT=ktile[DH:HC, :], rhs=QT[qb][DH:HC, :],
                    start=True, stop=True,
                )
                stiles[kt] = s

            def emit_exp(kt):
                s = stiles.pop(kt)
                ex = epool.tile([128, 1024], MMDT, tag="ex")
                if kt % 2 == 0:
                    nc.scalar.activation(
                        ex, s, mybir.ActivationFunctionType.Exp, scale=0.125,
                    )
                else:
                    nc.vector.tensor_scalar(
                        out=ex.bitcast(I16), in0=s,
                        scalar1=EXP_A, scalar2=EXP_B,
                        op0=mybir.AluOpType.mult, op1=mybir.AluOpType.add,
                    )
                extiles[kt] = ex

            def emit_pv(kt):
                ex = extiles.pop(kt)
                first, last = kt == 0, kt == NKT - 1
                nc.tensor.matmul(
                    pv0, lhsT=V2[:, kt, 0:65], rhs=ex[:, 0:512],
                    start=first, stop=last,
                )
                nc.tensor.matmul(
                    pv1, lhsT=V2[:, kt, 65:130], rhs=ex[:, 512:1024],
                    start=first, stop=last,
                )

            for kt in range(NKT + 2):
                if kt < NKT:
                    emit_qk(kt)
                if tail3 and 1 <= kt <= len(tail3):
                    tail3[kt - 1]()
                if kt >= 1 and kt - 1 < NKT:
                    emit_exp(kt - 1)
                if kt >= 2:
                    emit_pv(kt - 2)
            st = {"pvcs": [], "rsps": [], "qb": qb}
            for h, pv in ((0, pv0), (1, pv1)):
                pvc = rpool.tile([65, 512], F32, tag=f"pvc{h}", name="pvc")
                nc.vector.tensor_copy(pvc, pv)
                # l-row [1,512] -> dram -> [128,4] partition spread
                scr = dpool.tile([1, 512], F32, tag=f"scr{h}")
                nc.sync.dma_start(out=scr, in_=pvc[64:65, :])
                rsp = rpool.tile([128, 4], F32, tag=f"rsp{h}")
                nc.sync.dma_start(
                    out=rsp,
                    in_=bass.AP(tensor=scr.tensor, offset=scr.offset,
                                ap=[[1, 128], [128, 4]]),
                )
                st["pvcs"].append(pvc)
                st["rsps"].append(rsp)
            return st

        def phase2(st):
            st["bcs"] = []
            for h in range(2):
                rsp2 = rpool.tile([128, 4], F32, tag=f"rsp2{h}")
                nc.vector.reciprocal(rsp2, st["rsps"][h])
                scr2 = dpool.tile([1, 512], F32, tag=f"scr2{h}")
                nc.sync.dma_start(
                    out=bass.AP(tensor=scr2.tensor, offset=scr2.offset,
                                ap=[[1, 128], [128, 4]]),
                    in_=rsp2,
                )
                bc = rpool.tile([DH, 512], F32, tag=f"bc{h}")
                nc.sync.dma_start(
                    out=bc,
                    in_=bass.AP(tensor=scr2.tensor, offset=scr2.offset,
                                ap=[[0, DH]] + list(scr2.ap)[1:]),
                )
                st["bcs"].append(bc)

        def phase3(st):
            """Return a list of closures: ATT normalize then 8x out-proj
            (matmul + ACT evacuation + store); the caller spreads them
            through the next block's kt loop as filler."""
            qb = st["qb"]
            ATT = apool.tile([128, 512], MMDT, tag="att")

            def norm():
                for h in range(2):
                    hs = slice(h * DH, (h + 1) * DH)
                    nc.vector.tensor_mul(ATT[hs, :], st["pvcs"][h][0:DH, :],
                                         st["bcs"][h])

            def proj(qs, ec):
                def go():
                    po = opsum.tile([128, 512], F32, tag="pt_po")
                    nc.tensor.matmul(
                        po,
                        lhsT=ATT[:, qs * 128:(qs + 1) * 128],
                        rhs=wo_sb[:, ec * 512:(ec + 1) * 512],
                        start=True, stop=True,
                    )
                    osb = apool.tile([128, 512], F32, tag="osb")
                    nc.scalar.copy(osb, po)
                    nc.sync.dma_start(
                        out=out[qb * 512 + qs * 128:qb * 512 + (qs + 1) * 128,
                                ec * 512:(ec + 1) * 512],
                        in_=osb,
                    )
                return go

            return [norm] + [proj(qs, ec) for qs in range(NQS) for ec in range(NEC)]

        pending = []
        tail3 = None
        for qb in range(NSC):
            st = phase1(qb, tail3)
            tail3 = None
            pending.append(st)
            if len(pending) >= 2:
                phase2(pending[-2])
            if len(pending) >= 3:
                tail3 = phase3(pending.pop(0))
        phase2(pending[-1])
        if tail3:
            for fn in tail3:
                fn()
        for fn in phase3(pending.pop(0)):
            fn()
        for fn in phase3(pending.pop(0)):
            fn()
    nc.finalize()
    return nc


def _get_nc(S=SEQ):
    key = S
    if key not in _NC_CACHE:
        _NC_CACHE[key] = _build_nc(S=S)
    return _NC_CACHE[key]


def _make_in_maps(x, Wq, bq, Wk, bk, Wv, bv, Wo, npdt=np.float16):
    xT = np.ascontiguousarray(np.asarray(x, np.float32)[0].T.astype(npdt))
    Wq, Wk, Wv, Wo = (np.asarray(a, np.float32).astype(npdt) for a in (Wq, Wk, Wv, Wo))
    bq, bk, bv = (np.asarray(a, np.float32) for a in (bq, bk, bv))
    in_maps = []
    for c in range(N_CORES):
        sl = slice(c * HC, (c + 1) * HC)
        in_maps.append({
            "xT": xT,
            "wq": np.ascontiguousarray(Wq[:, sl]),
            "wk": np.ascontiguousarray(Wk[:, sl]),
            "wv": np.ascontiguousarray(Wv[:, sl]),
            "bq": np.ascontiguousarray(bq[sl]).reshape(HC, 1),
            "bk": np.ascontiguousarray(bk[sl]).reshape(HC, 1),
            "bv": np.ascontiguousarray(bv[sl]).reshape(HC, 1),
            "wo": np.ascontiguousarray(Wo[sl, :]),
        })
    return in_maps


def run(inputs, trace=False, mmdt="fp16"):
    """Run the kernel; returns (out [1,S,E] float32, BassKernelResults)."""
    from concourse.bass_utils import run_bass_kernel_spmd

    nc = _get_nc()
    in_maps = _make_in_maps(
        inputs["x"], inputs["Wq"], inputs["bq"], inputs["Wk"], inputs["bk"],
        inputs["Wv"], inputs["bv"], inputs["Wo"],
    )
    res = run_bass_kernel_spmd(
        nc, in_maps, core_ids=list(range(N_CORES)), trace=trace
    )
    acc = np.zeros((SEQ, EMBED), np.float64)
    for c in range(N_CORES):
        acc += res.results[c]["out"]
    acc += np.asarray(inputs["bo"], np.float64)
    return acc.astype(np.float32).reshape(1, SEQ, EMBED), res


def kernel(x, Wq, bq, Wk, bk, Wv, bv, Wo, bo):
    out, _ = run(dict(x=x, Wq=Wq, bq=bq, Wk=Wk, bk=bk, Wv=Wv, bv=bv, Wo=Wo, bo=bo))
    return out


# revision 19
# speedup vs baseline: 1.3978x; 1.0080x over previous
"""TRN2 Bass/Tile kernel: 16-head MHA, B=1 S=4096 E=1024, head-sharded over 8 cores.

Sharding: tensor-parallel over heads. Core c owns heads {2c, 2c+1}: columns
[128c, 128(c+1)) of Wq/Wk/Wv (+bias slices) and rows [128c, 128(c+1)) of Wo.
Each core computes attention for its 2 heads and a partial out-projection
[S, E]; the host sums the 8 partials and adds bo (TP row-parallel unshard).

v2 design (PE-queue-bound baseline was 464us: MATMUL 336 + serial LDWEIGHTS 126):
  A) QT/KT/VT [128ch, S] = W_c^T @ x^T   (lhsT=W-slice, rhs=xT tiles, +bias on DVE)
     KT natural (no per-head zero-padding).
  B) V2 [128k, kt, 130] = [V_h0|ones|V_h1|ones] via PE transpose (l-sum ride-along)
  C) per 512-q block, per key-tile kt:
     - QK row-tiled: two concurrent K=64 matmuls (tile_position (0,0)/(64,0))
       -> scores^T [128k, 1024] = [h0 512q | h1 512q] in one PSUM pair
     - exp split by kt parity: even kt on ACT (true exp, scale=1/8), odd kt on
       DVE via Schraudolph fp16-bit trick: int16 = round(a*s + b) bitcast fp16
       (a = 1024*log2e/8, b = 15*1024 + C). Softmax renormalization absorbs
       the approximation's constant factor; mixed-tile error ~5e-3 rel (sim).
     - PV accumulate psum[65, 512] per head: rows 0:64 attn^T, row 64 = denom l
  D) per q-block: recip(l) via DRAM partition-spread, ATT = attn^T * (1/l),
     out-proj [q, E] with PSUM evacuation split ACT/DVE by parity.
"""

import sys

for _p in ("/opt/trn_rl_repo", "/opt/pypackages"):
    if _p not in sys.path:
        sys.path.append(_p)

import numpy as np

EMBED = 1024
N_CORES = 8
HC = EMBED // N_CORES  # 128 channels = 2 heads per core
DH = 64                # head dim
SEQ = 4096

# Schraudolph fp16 exp constants: exp(s/8) ~= bits_fp16(round(EXP_A*s + EXP_B))
EXP_A = 1024.0 * 1.4426950408889634 * 0.125
EXP_B = 15.0 * 1024.0 - 45.0

_NC_CACHE = {}


def _build_nc(S=SEQ, E=EMBED):
    from contextlib import ExitStack

    import concourse.bass as bass
    import concourse.mybir as mybir
    import concourse.tile as tile
    from concourse import bacc
    from concourse.masks import make_identity

    F32 = mybir.dt.float32
    MMDT = mybir.dt.float16
    I16 = mybir.dt.int16

    ET = E // 128      # 8 E-tiles of 128 (contraction for projections)
    NSC = S // 512     # 8 512-wide S chunks
    NKT = S // 128     # 32 128-wide key tiles
    NQS = 512 // 128   # 4 128-q subtiles per chunk
    NEC = E // 512     # 2 512-wide E chunks of the out-projection

    nc = bacc.Bacc()
    xT = nc.declare_dram_parameter("xT", [E, S], MMDT, isOutput=False)
    wq = nc.declare_dram_parameter("wq", [E, HC], MMDT, isOutput=False)
    wk = nc.declare_dram_parameter("wk", [E, HC], MMDT, isOutput=False)
    wv = nc.declare_dram_parameter("wv", [E, HC], MMDT, isOutput=False)
    bq = nc.declare_dram_parameter("bq", [HC, 1], F32, isOutput=False)
    bk = nc.declare_dram_parameter("bk", [HC, 1], F32, isOutput=False)
    bv = nc.declare_dram_parameter("bv", [HC, 1], F32, isOutput=False)
    wo = nc.declare_dram_parameter("wo", [HC, E], MMDT, isOutput=False)
    out = nc.declare_dram_parameter("out", [S, E], F32, isOutput=True)

    with tile.TileContext(nc) as tc, ExitStack() as ctx:
        wpool = ctx.enter_context(tc.tile_pool(name="w", bufs=1))
        xpool = ctx.enter_context(tc.tile_pool(name="x", bufs=8))
        qkvpool = ctx.enter_context(tc.tile_pool(name="qkv", bufs=1))
        v2pool = ctx.enter_context(tc.tile_pool(name="v2", bufs=1))
        epool = ctx.enter_context(tc.tile_pool(name="e", bufs=6))
        apool = ctx.enter_context(tc.tile_pool(name="a", bufs=3))
        rpool = ctx.enter_context(tc.tile_pool(name="r", bufs=4))
        dpool = ctx.enter_context(tc.tile_pool(name="d", bufs=4, space="DRAM"))
        # PSUM 8 banks: scores double-buffer 2x[128,1024]=4, PV 2x[65,512]=2,
        # transpose/out-proj 2x[128,512]=2
        spsum = ctx.enter_context(tc.tile_pool(name="sp", bufs=2, space="PSUM"))
        pvpsum = ctx.enter_context(tc.tile_pool(name="pv", bufs=2, space="PSUM"))
        opsum = ctx.enter_context(tc.tile_pool(name="op", bufs=2, space="PSUM"))

        # --- weights / constants ---
        w_sb = {}
        for name, src in (("wq", wq), ("wk", wk), ("wv", wv)):
            t = wpool.tile([128, ET, HC], MMDT, tag=name, name=name)
            nc.sync.dma_start(out=t, in_=src.rearrange("(a p) c -> p a c", p=128))
            w_sb[name] = t
        wo_sb = wpool.tile([HC, E], MMDT, tag="wo")
        nc.sync.dma_start(out=wo_sb, in_=wo[:, :])
        b_sb = {}
        for name, src in (("bq", bq), ("bk", bk), ("bv", bv)):
            t = wpool.tile([HC, 1], F32, tag=name, name=name)
            nc.sync.dma_start(out=t, in_=src[:, :])
            b_sb[name] = t
        ident = wpool.tile([128, 128], MMDT, tag="ident")
        make_identity(nc, ident)

        # --- stage A: QT/KT/VT [128ch, S] chunked by 512, natural head layout ---
        QT = [qkvpool.tile([HC, 512], MMDT, tag=f"qt{i}", name=f"qt{i}") for i in range(NSC)]
        KT = [qkvpool.tile([HC, 512], MMDT, tag=f"kt{i}", name=f"kt{i}") for i in range(NSC)]
        VT = [qkvpool.tile([HC, 512], MMDT, tag=f"vt{i}", name=f"vt{i}") for i in range(NSC)]
        for sc in range(NSC):
            big1 = spsum.tile([128, 1024], F32, tag="big")
            big2 = opsum.tile([128, 512], F32, tag="pt_po")
            for et in range(ET):
                xt = xpool.tile([128, 512], MMDT, tag="xt")
                nc.sync.dma_start(
                    out=xt, in_=xT[et * 128:(et + 1) * 128, sc * 512:(sc + 1) * 512]
                )
                first, last = et == 0, et == ET - 1
                nc.tensor.matmul(big1[:, 0:512], lhsT=w_sb["wq"][:, et, :],
                                 rhs=xt, start=first, stop=last)
                nc.tensor.matmul(big1[:, 512:1024], lhsT=w_sb["wk"][:, et, :],
                                 rhs=xt, start=first, stop=last)
                nc.tensor.matmul(big2[:, 0:512], lhsT=w_sb["wv"][:, et, :],
                                 rhs=xt, start=first, stop=last)
            nc.vector.tensor_scalar_add(QT[sc], big1[:, 0:512], b_sb["bq"])
            nc.vector.tensor_scalar_add(KT[sc], big1[:, 512:1024], b_sb["bk"])
            nc.vector.tensor_scalar_add(VT[sc], big2[:, 0:512], b_sb["bv"])

        # --- stage B: V2 [128k, NKT, 195] = [V_h0|ones|V_h1|ones|zero-pad] ---
        # (PV lhsT is padded to M=128 — full-width stationary loads measure
        # faster than M=65 — so each head's slice drags in 63 junk columns
        # whose PSUM rows are simply never read; the pad keeps them finite.)
        V2 = v2pool.tile([128, NKT, 195], MMDT, tag="V2")
        nc.vector.memset(V2[:, :, 64:65], 1.0)
        nc.vector.memset(V2[:, :, 129:130], 1.0)
        nc.vector.memset(V2[:, :, 130:195], 0.0)
        for kt in range(NKT):
            pt = opsum.tile([128, 512], MMDT, tag="pt_po")
            nc.tensor.transpose(
                pt[:, 0:128], VT[kt // 4][:, (kt % 4) * 128:(kt % 4 + 1) * 128], ident
            )
            # one copy: [128, 2, 64] -> V2 cols {0:64, 65:129}
            dst = bass.AP(
                tensor=V2.tensor, offset=V2.offset + kt * 195,
                ap=[list(V2.ap[0]), [65, 2], [1, 64]],
            )
            src = bass.AP(
                tensor=pt.tensor, offset=pt.offset,
                ap=[list(pt.ap[0]), [64, 2], [1, 64]],
            )
            nc.vector.tensor_copy(dst, src)

        # --- stage C+D: per 512-q block over all 32 key tiles, both heads.
        # Stage D is split into three phases, each deferred one q-block
        # behind, so neither the strict-FIFO ACT/DVE queues nor the PE queue
        # ever wait on the reciprocal's DRAM round-trips:
        #   phase1(qb): kt loop + PV-psum evacuation + l-row spread DMAs
        #   phase2(qb): reciprocal + spread-back DMAs     (at end of qb+1)
        #   phase3(qb): ATT scale + out-proj + store      (at end of qb+2)
        def phase1(qb, tail3):
            """kt loop, software-pipelined emission: QK(kt) | exp(kt-1) |
            PV(kt-2), with the previous blocks' phase2/phase3 work (tail3)
            spread through the early iterations as PE/ACT/DVE filler."""
            pv0 = pvpsum.tile([128, 512], F32, tag="pv", name="pv0")
            pv1 = pvpsum.tile([128, 512], F32, tag="pv", name="pv1")
            stiles = {}
            extiles = {}

            def emit_qk(kt):
                s = spsum.tile([128, 1024], F32, tag="big")
                ktile = KT[kt // 4][:, (kt % 4) * 128:(kt % 4 + 1) * 128]
                nc.tensor.matmul(
                    s[:, 0:512], lhsT=ktile[0:DH, :], rhs=QT[qb][0:DH, :],
                    start=True, stop=True,
                )
                nc.tensor.matmul(
                    s[:, 512:1024], lhsT=ktile[DH:HC, :], rhs=QT[qb][DH:HC, :],
                    start=True, stop=True,
                )
                stiles[kt] = s

            def emit_exp(kt):
                s = stiles.pop(kt)
                ex = epool.tile([128, 1024], MMDT, tag="ex")
                if kt % 2 == 0:
                    nc.scalar.activation(
                        ex, s, mybir.ActivationFunctionType.Exp, scale=0.125,
                    )
                else:
                    nc.vector.tensor_scalar(
                        out=ex.bitcast(I16), in0=s,
                        scalar1=EXP_A, scalar2=EXP_B,
                        op0=mybir.AluOpType.mult, op1=mybir.AluOpType.add,
                    )
                extiles[kt] = ex

            def emit_pv(kt):
                ex = extiles.pop(kt)
                first, last = kt == 0, kt == NKT - 1
                nc.tensor.matmul(
                    pv0, lhsT=V2[:, kt, 0:128], rhs=ex[:, 0:512],
                    start=first, stop=last,
                )
                nc.tensor.matmul(
                    pv1, lhsT=V2[:, kt, 65:193], rhs=ex[:, 512:1024],
                    start=first, stop=last,
                )

            toff = max(1, 12 - len(tail3)) if tail3 else 0
            for kt in range(NKT + 2):
                if kt < NKT:
                    emit_qk(kt)
                if tail3 and toff <= kt < toff + len(tail3):
                    tail3[kt - toff]()
                if kt >= 1 and kt - 1 < NKT:
                    emit_exp(kt - 1)
                if kt >= 2:
                    emit_pv(kt - 2)
            st = {"pvcs": [], "rsps": [], "bcs": [None, None], "qb": qb}
            for h, pv in ((0, pv0), (1, pv1)):
                pvc = rpool.tile([65, 512], F32, tag=f"pvc{h}", name="pvc")
                if h == 0:
                    nc.scalar.copy(pvc, pv[0:65, :])
                else:
                    nc.vector.tensor_copy(pvc, pv[0:65, :])
                # l-row [1,512] -> dram -> [128,4] partition spread
                scr = dpool.tile([1, 512], F32, tag=f"scr{h}")
                nc.sync.dma_start(out=scr, in_=pvc[64:65, :])
                rsp = rpool.tile([128, 4], F32, tag=f"rsp{h}")
                nc.sync.dma_start(
                    out=rsp,
                    in_=bass.AP(tensor=scr.tensor, offset=scr.offset,
                                ap=[[1, 128], [128, 4]]),
                )
                st["pvcs"].append(pvc)
                st["rsps"].append(rsp)
            return st

        def phase2(st):
            """Return one closure per head: reciprocal + spread-back DMAs."""
            def go(h):
                def run():
                    rsp2 = rpool.tile([128, 4], F32, tag=f"rsp2{h}")
                    nc.vector.reciprocal(rsp2, st["rsps"][h])
                    scr2 = dpool.tile([1, 512], F32, tag=f"scr2{h}")
                    nc.sync.dma_start(
                        out=bass.AP(tensor=scr2.tensor, offset=scr2.offset,
                                    ap=[[1, 128], [128, 4]]),
                        in_=rsp2,
                    )
                    bc = rpool.tile([DH, 512], F32, tag=f"bc{h}")
                    nc.sync.dma_start(
                        out=bc,
                        in_=bass.AP(tensor=scr2.tensor, offset=scr2.offset,
                                    ap=[[0, DH]] + list(scr2.ap)[1:]),
                    )
                    st["bcs"][h] = bc
                return run
            return [go(0), go(1)]

        def phase3(st, alt=False):
            """Return a list of closures: ATT normalize then 8x out-proj
            (matmul + evacuation + store); the caller spreads them through
            the next block's kt loop as filler. alt=True alternates the
            evacuation engine (for the final drain when both are idle)."""
            qb = st["qb"]
            ATT = apool.tile([128, 512], MMDT, tag="att")

            def norm():
                for h in range(2):
                    hs = slice(h * DH, (h + 1) * DH)
                    nc.vector.tensor_mul(ATT[hs, :], st["pvcs"][h][0:DH, :],
                                         st["bcs"][h])

            def proj(qs, ec):
                def go():
                    po = opsum.tile([128, 512], F32, tag="pt_po")
                    nc.tensor.matmul(
                        po,
                        lhsT=ATT[:, qs * 128:(qs + 1) * 128],
                        rhs=wo_sb[:, ec * 512:(ec + 1) * 512],
                        start=True, stop=True,
                    )
                    osb = apool.tile([128, 512], F32, tag="osb")
                    if alt and (qs * NEC + ec) % 2 == 0:
                        nc.vector.tensor_copy(osb, po)
                    else:
                        nc.scalar.copy(osb, po)
                    nc.sync.dma_start(
                        out=out[qb * 512 + qs * 128:qb * 512 + (qs + 1) * 128,
                                ec * 512:(ec + 1) * 512],
                        in_=osb,
                    )
                return go

            return [norm] + [proj(qs, ec) for qs in range(NQS) for ec in range(NEC)]

        pending = []
        ph2_prev = None
        for qb in range(NSC):
            tail3 = []
            if len(pending) >= 2:
                tail3 += phase3(pending.pop(0))
            if ph2_prev:
                tail3 += ph2_prev
            st = phase1(qb, tail3 or None)
            ph2_prev = phase2(st)
            pending.append(st)
        for fn in ph2_prev:
            fn()
        for fn in phase3(pending.pop(0), alt=True):
            fn()
        for fn in phase3(pending.pop(0), alt=True):
            fn()
    nc.finalize()
    return nc


def _get_nc(S=SEQ):
    key = S
    if key not in _NC_CACHE:
        _NC_CACHE[key] = _build_nc(S=S)
    return _NC_CACHE[key]


def _make_in_maps(x, Wq, bq, Wk, bk, Wv, bv, Wo, npdt=np.float16):
    xT = np.ascontiguousarray(np.asarray(x, np.float32)[0].T.astype(npdt))
    Wq, Wk, Wv, Wo = (np.asarray(a, np.float32).astype(npdt) for a in (Wq, Wk, Wv, Wo))
    bq, bk, bv = (np.asarray(a, np.float32) for a in (bq, bk, bv))
    in_maps = []
    for c in range(N_CORES):
        sl = slice(c * HC, (c + 1) * HC)
        in_maps.append({
            "xT": xT,
            "wq": np.ascontiguousarray(Wq[:, sl]),
            "wk": np.ascontiguousarray(Wk[:, sl]),
            "wv": np.ascontiguousarray(Wv[:, sl]),
            "bq": np.ascontiguousarray(bq[sl]).reshape(HC, 1),
            "bk": np.ascontiguousarray(bk[sl]).reshape(HC, 1),
            "bv": np.ascontiguousarray(bv[sl]).reshape(HC, 1),
            "wo": np.ascontiguousarray(Wo[sl, :]),
        })
    return in_maps


def run(inputs, trace=False, mmdt="fp16"):
    """Run the kernel; returns (out [1,S,E] float32, BassKernelResults)."""
    from concourse.bass_utils import run_bass_kernel_spmd

    nc = _get_nc()
    in_maps = _make_in_maps(
        inputs["x"], inputs["Wq"], inputs["bq"], inputs["Wk"], inputs["bk"],
        inputs["Wv"], inputs["bv"], inputs["Wo"],
    )
    res = run_bass_kernel_spmd(
        nc, in_maps, core_ids=list(range(N_CORES)), trace=trace
    )
    acc = np.zeros((SEQ, EMBED), np.float64)
    for c in range(N_CORES):
        acc += res.results[c]["out"]
    acc += np.asarray(inputs["bo"], np.float64)
    return acc.astype(np.float32).reshape(1, SEQ, EMBED), res


def kernel(x, Wq, bq, Wk, bk, Wv, bv, Wo, bo):
    out, _ = run(dict(x=x, Wq=Wq, bq=bq, Wk=Wk, bk=bk, Wv=Wv, bv=bv, Wo=Wo, bo=bo))
    return out


# revision 30
# speedup vs baseline: 1.4583x; 1.0432x over previous
"""TRN2 Bass/Tile kernel: 16-head MHA, B=1 S=4096 E=1024, head-sharded over 8 cores.

Sharding: tensor-parallel over heads. Core c owns heads {2c, 2c+1}: columns
[128c, 128(c+1)) of Wq/Wk/Wv (+bias slices) and rows [128c, 128(c+1)) of Wo.
Each core computes attention for its 2 heads and a partial out-projection
[S, E]; the host sums the 8 partials and adds bo (TP row-parallel unshard).

v2 design (PE-queue-bound baseline was 464us: MATMUL 336 + serial LDWEIGHTS 126):
  A) QT/KT/VT [128ch, S] = W_c^T @ x^T   (lhsT=W-slice, rhs=xT tiles, +bias on DVE)
     KT natural (no per-head zero-padding).
  B) V2 [128k, kt, 130] = [V_h0|ones|V_h1|ones] via PE transpose (l-sum ride-along)
  C) per 512-q block, per key-tile kt:
     - QK row-tiled: two concurrent K=64 matmuls (tile_position (0,0)/(64,0))
       -> scores^T [128k, 1024] = [h0 512q | h1 512q] in one PSUM pair
     - exp split by kt parity: even kt on ACT (true exp, scale=1/8), odd kt on
       DVE via Schraudolph fp16-bit trick: int16 = round(a*s + b) bitcast fp16
       (a = 1024*log2e/8, b = 15*1024 + C). Softmax renormalization absorbs
       the approximation's constant factor; mixed-tile error ~5e-3 rel (sim).
     - PV accumulate psum[65, 512] per head: rows 0:64 attn^T, row 64 = denom l
  D) per q-block, software-pipelined across the next blocks' kt loops:
     1/l via DVE reciprocal_approx_fast on the [1,512] l-row, partition-
     broadcast via a K=1 ones outer-product matmul (no DMA round trips),
     ATT = attn^T * (1/l), out-proj [q, E], evacuations split ACT/DVE.
"""

import sys

for _p in ("/opt/trn_rl_repo", "/opt/pypackages"):
    if _p not in sys.path:
        sys.path.append(_p)

import numpy as np

EMBED = 1024
N_CORES = 8
HC = EMBED // N_CORES  # 128 channels = 2 heads per core
DH = 64                # head dim
SEQ = 4096

# Schraudolph fp16 exp constants: exp(s/8) ~= bits_fp16(round(EXP_A*s + EXP_B))
EXP_A = 1024.0 * 1.4426950408889634 * 0.125
EXP_B = 15.0 * 1024.0 - 45.0

_NC_CACHE = {}


def _build_nc(S=SEQ, E=EMBED):
    from contextlib import ExitStack

    import concourse.bass as bass
    import concourse.mybir as mybir
    import concourse.tile as tile
    from concourse import bacc
    from concourse.masks import make_identity

    F32 = mybir.dt.float32
    MMDT = mybir.dt.float16
    I16 = mybir.dt.int16

    ET = E // 128      # 8 E-tiles of 128 (contraction for projections)
    NSC = S // 512     # 8 512-wide S chunks
    NKT = S // 128     # 32 128-wide key tiles
    NQS = 512 // 128   # 4 128-q subtiles per chunk
    NEC = E // 512     # 2 512-wide E chunks of the out-projection

    nc = bacc.Bacc()
    xT = nc.declare_dram_parameter("xT", [E, S], MMDT, isOutput=False)
    wq = nc.declare_dram_parameter("wq", [E, HC], MMDT, isOutput=False)
    wk = nc.declare_dram_parameter("wk", [E, HC], MMDT, isOutput=False)
    wv = nc.declare_dram_parameter("wv", [E, HC], MMDT, isOutput=False)
    bq = nc.declare_dram_parameter("bq", [HC, 1], F32, isOutput=False)
    bk = nc.declare_dram_parameter("bk", [HC, 1], F32, isOutput=False)
    bv = nc.declare_dram_parameter("bv", [HC, 1], F32, isOutput=False)
    wo = nc.declare_dram_parameter("wo", [HC, E], MMDT, isOutput=False)
    out = nc.declare_dram_parameter("out", [S, E], F32, isOutput=True)

    with tile.TileContext(nc) as tc, ExitStack() as ctx:
        wpool = ctx.enter_context(tc.tile_pool(name="w", bufs=1))
        xpool = ctx.enter_context(tc.tile_pool(name="x", bufs=8))
        qkvpool = ctx.enter_context(tc.tile_pool(name="qkv", bufs=1))
        v2pool = ctx.enter_context(tc.tile_pool(name="v2", bufs=1))
        epool = ctx.enter_context(tc.tile_pool(name="e", bufs=6))
        apool = ctx.enter_context(tc.tile_pool(name="a", bufs=3))
        rpool = ctx.enter_context(tc.tile_pool(name="r", bufs=4))
        # PSUM 8 banks: scores double-buffer 2x[128,1024]=4, PV 2x[65,512]=2,
        # transpose/out-proj 2x[128,512]=2
        spsum = ctx.enter_context(tc.tile_pool(name="sp", bufs=2, space="PSUM"))
        pvpsum = ctx.enter_context(tc.tile_pool(name="pv", bufs=2, space="PSUM"))
        opsum = ctx.enter_context(tc.tile_pool(name="op", bufs=2, space="PSUM"))

        # --- weights / constants ---
        w_sb = {}
        for name, src in (("wq", wq), ("wk", wk), ("wv", wv)):
            t = wpool.tile([128, ET, HC], MMDT, tag=name, name=name)
            nc.sync.dma_start(out=t, in_=src.rearrange("(a p) c -> p a c", p=128))
            w_sb[name] = t
        wo_sb = wpool.tile([HC, E], MMDT, tag="wo")
        nc.sync.dma_start(out=wo_sb, in_=wo[:, :])
        b_sb = {}
        for name, src in (("bq", bq), ("bk", bk), ("bv", bv)):
            t = wpool.tile([HC, 1], F32, tag=name, name=name)
            nc.sync.dma_start(out=t, in_=src[:, :])
            b_sb[name] = t
        ident = wpool.tile([128, 128], MMDT, tag="ident")
        make_identity(nc, ident)
        ones1 = wpool.tile([1, DH], MMDT, tag="ones1")
        nc.vector.memset(ones1, 1.0)

        # --- stage A: QT/KT/VT [128ch, S] chunked by 512, natural head layout ---
        QT = [qkvpool.tile([HC, 512], MMDT, tag=f"qt{i}", name=f"qt{i}") for i in range(NSC)]
        KT = [qkvpool.tile([HC, 512], MMDT, tag=f"kt{i}", name=f"kt{i}") for i in range(NSC)]
        VT = [qkvpool.tile([HC, 512], MMDT, tag=f"vt{i}", name=f"vt{i}") for i in range(NSC)]
        for sc in range(NSC):
            big1 = spsum.tile([128, 1024], F32, tag="big")
            big2 = opsum.tile([128, 512], F32, tag="pt_po")
            for et in range(ET):
                xt = xpool.tile([128, 512], MMDT, tag="xt")
                nc.sync.dma_start(
                    out=xt, in_=xT[et * 128:(et + 1) * 128, sc * 512:(sc + 1) * 512]
                )
                first, last = et == 0, et == ET - 1
                nc.tensor.matmul(big1[:, 0:512], lhsT=w_sb["wq"][:, et, :],
                                 rhs=xt, start=first, stop=last)
                nc.tensor.matmul(big1[:, 512:1024], lhsT=w_sb["wk"][:, et, :],
                                 rhs=xt, start=first, stop=last)
                nc.tensor.matmul(big2[:, 0:512], lhsT=w_sb["wv"][:, et, :],
                                 rhs=xt, start=first, stop=last)
            nc.vector.tensor_scalar_add(QT[sc], big1[:, 0:512], b_sb["bq"])
            nc.vector.tensor_scalar_add(KT[sc], big1[:, 512:1024], b_sb["bk"])
            nc.vector.tensor_scalar_add(VT[sc], big2[:, 0:512], b_sb["bv"])

        # --- stage B: V2 [128k, NKT, 195] = [V_h0|ones|V_h1|ones|zero-pad] ---
        # (PV lhsT is padded to M=128 — full-width stationary loads measure
        # faster than M=65 — so each head's slice drags in 63 junk columns
        # whose PSUM rows are simply never read; the pad keeps them finite.)
        V2 = v2pool.tile([128, NKT, 195], MMDT, tag="V2")
        nc.vector.memset(V2[:, :, 64:65], 1.0)
        nc.vector.memset(V2[:, :, 129:130], 1.0)
        nc.vector.memset(V2[:, :, 130:195], 0.0)
        for kt in range(NKT):
            pt = opsum.tile([128, 512], MMDT, tag="pt_po")
            nc.tensor.transpose(
                pt[:, 0:128], VT[kt // 4][:, (kt % 4) * 128:(kt % 4 + 1) * 128], ident
            )
            # one copy: [128, 2, 64] -> V2 cols {0:64, 65:129}
            dst = bass.AP(
                tensor=V2.tensor, offset=V2.offset + kt * 195,
                ap=[list(V2.ap[0]), [65, 2], [1, 64]],
            )
            src = bass.AP(
                tensor=pt.tensor, offset=pt.offset,
                ap=[list(pt.ap[0]), [64, 2], [1, 64]],
            )
            nc.vector.tensor_copy(dst, src)

        # --- stage C+D: per 512-q block over all 32 key tiles, both heads.
        # Stage D is split into three phases, each deferred one q-block
        # behind, so neither the strict-FIFO ACT/DVE queues nor the PE queue
        # ever wait on the reciprocal's DRAM round-trips:
        #   phase1(qb): kt loop + PV-psum evacuation + l-row spread DMAs
        #   phase2(qb): reciprocal + spread-back DMAs     (at end of qb+1)
        #   phase3(qb): ATT scale + out-proj + store      (at end of qb+2)
        def phase1(qb, tail3):
            """kt loop, software-pipelined emission: QK(kt) | exp(kt-1) |
            PV(kt-2), with the previous blocks' phase2/phase3 work (tail3)
            spread through the early iterations as PE/ACT/DVE filler."""
            pv0 = pvpsum.tile([128, 512], F32, tag="pv", name="pv0")
            pv1 = pvpsum.tile([128, 512], F32, tag="pv", name="pv1")
            stiles = {}
            extiles = {}

            def emit_qk(kt):
                s = spsum.tile([128, 1024], F32, tag="big")
                ktile = KT[kt // 4][:, (kt % 4) * 128:(kt % 4 + 1) * 128]
                nc.tensor.matmul(
                    s[:, 0:512], lhsT=ktile[0:DH, :], rhs=QT[qb][0:DH, :],
                    start=True, stop=True,
                )
                nc.tensor.matmul(
                    s[:, 512:1024], lhsT=ktile[DH:HC, :], rhs=QT[qb][DH:HC, :],
                    start=True, stop=True,
                )
                stiles[kt] = s

            def emit_exp(kt):
                s = stiles.pop(kt)
                ex = epool.tile([128, 1024], MMDT, tag="ex")
                if kt % 2 == 0:
                    nc.scalar.activation(
                        ex, s, mybir.ActivationFunctionType.Exp, scale=0.125,
                    )
                else:
                    nc.vector.tensor_scalar(
                        out=ex.bitcast(I16), in0=s,
                        scalar1=EXP_A, scalar2=EXP_B,
                        op0=mybir.AluOpType.mult, op1=mybir.AluOpType.add,
                    )
                extiles[kt] = ex

            def emit_pv(kt):
                ex = extiles.pop(kt)
                first, last = kt == 0, kt == NKT - 1
                nc.tensor.matmul(
                    pv0, lhsT=V2[:, kt, 0:128], rhs=ex[:, 0:512],
                    start=first, stop=last,
                )
                nc.tensor.matmul(
                    pv1, lhsT=V2[:, kt, 65:193], rhs=ex[:, 512:1024],
                    start=first, stop=last,
                )

            toff = max(1, 12 - len(tail3)) if tail3 else 0
            for kt in range(NKT + 2):
                if kt < NKT:
                    emit_qk(kt)
                if tail3 and toff <= kt < toff + len(tail3):
                    tail3[kt - toff]()
                if kt >= 1 and kt - 1 < NKT:
                    emit_exp(kt - 1)
                if kt >= 2:
                    emit_pv(kt - 2)
            st = {"pvcs": [], "qb": qb}
            for h, pv in ((0, pv0), (1, pv1)):
                pvc = rpool.tile([65, 512], F32, tag=f"pvc{h}", name="pvc")
                if h == 0:
                    nc.scalar.copy(pvc, pv[0:65, :])
                else:
                    nc.vector.tensor_copy(pvc, pv[0:65, :])
                st["pvcs"].append(pvc)
            return st

        def phase2(st):
            """Per head: 1/l on the [1,512] row (DVE approx), partition-
            broadcast via a K=1 ones outer-product on PE, normalize into ATT.
            All on-chip -- no DMA round trips."""
            ATT = apool.tile([128, 512], MMDT, tag="att")
            st["ATT"] = ATT

            def go(h):
                def run():
                    hs = slice(h * DH, (h + 1) * DH)
                    # 1/l = exp(-ln(l)) on ACT: both fns live in the
                    # natural_log_exp_and_others table set (no reload), and
                    # Exp converts straight to fp16 for the broadcast matmul.
                    rln = rpool.tile([1, 512], F32, tag=f"rln{h}")
                    nc.scalar.activation(
                        rln, st["pvcs"][h][64:65, :],
                        mybir.ActivationFunctionType.Ln,
                    )
                    rrowh = rpool.tile([1, 512], MMDT, tag=f"rrowh{h}")
                    nc.scalar.activation(
                        rrowh, rln, mybir.ActivationFunctionType.Exp,
                        scale=-1.0,
                    )
                    bcp = opsum.tile([128, 512], F32, tag="pt_po")
                    nc.tensor.matmul(
                        bcp[0:DH, :], lhsT=ones1, rhs=rrowh,
                        start=True, stop=True,
                    )
                    nc.vector.tensor_mul(ATT[hs, :], st["pvcs"][h][0:DH, :],
                                         bcp[0:DH, :])
                return run
            return [go(0), go(1)]

        def phase3(st, alt=False):
            """Return a list of closures: 8x out-proj (matmul + evacuation +
            store); the caller spreads them through the next block's kt loop
            as filler. alt=True alternates the evacuation engine (for the
            final drain when both are idle)."""
            qb = st["qb"]
            ATT = st["ATT"]

            def proj(qs, ec):
                def go():
                    po = opsum.tile([128, 512], F32, tag="pt_po")
                    nc.tensor.matmul(
                        po,
                        lhsT=ATT[:, qs * 128:(qs + 1) * 128],
                        rhs=wo_sb[:, ec * 512:(ec + 1) * 512],
                        start=True, stop=True,
                    )
                    osb = apool.tile([128, 512], F32, tag="osb")
                    if alt and (qs * NEC + ec) % 2 == 0:
                        nc.vector.tensor_copy(osb, po)
                    else:
                        nc.scalar.copy(osb, po)
                    nc.sync.dma_start(
                        out=out[qb * 512 + qs * 128:qb * 512 + (qs + 1) * 128,
                                ec * 512:(ec + 1) * 512],
                        in_=osb,
                    )
                return go

            return [proj(qs, ec) for qs in range(NQS) for ec in range(NEC)]

        pending = []
        ph2_prev = None
        for qb in range(NSC):
            tail3 = []
            if len(pending) >= 2:
                tail3 += phase3(pending.pop(0))
            if ph2_prev:
                tail3 += ph2_prev
            st = phase1(qb, tail3 or None)
            ph2_prev = phase2(st)
            pending.append(st)
        for fn in ph2_prev:
            fn()
        for fn in phase3(pending.pop(0), alt=True):
            fn()
        for fn in phase3(pending.pop(0), alt=True):
            fn()
    nc.finalize()
    return nc


def _get_nc(S=SEQ):
    key = S
    if key not in _NC_CACHE:
        _NC_CACHE[key] = _build_nc(S=S)
    return _NC_CACHE[key]


def _make_in_maps(x, Wq, bq, Wk, bk, Wv, bv, Wo, npdt=np.float16):
    xT = np.ascontiguousarray(np.asarray(x, np.float32)[0].T.astype(npdt))
    Wq, Wk, Wv, Wo = (np.asarray(a, np.float32).astype(npdt) for a in (Wq, Wk, Wv, Wo))
    bq, bk, bv = (np.asarray(a, np.float32) for a in (bq, bk, bv))
    in_maps = []
    for c in range(N_CORES):
        sl = slice(c * HC, (c + 1) * HC)
        in_maps.append({
            "xT": xT,
            "wq": np.ascontiguousarray(Wq[:, sl]),
            "wk": np.ascontiguousarray(Wk[:, sl]),
            "wv": np.ascontiguousarray(Wv[:, sl]),
            "bq": np.ascontiguousarray(bq[sl]).reshape(HC, 1),
            "bk": np.ascontiguousarray(bk[sl]).reshape(HC, 1),
            "bv": np.ascontiguousarray(bv[sl]).reshape(HC, 1),
            "wo": np.ascontiguousarray(Wo[sl, :]),
        })
    return in_maps


def run(inputs, trace=False, mmdt="fp16"):
    """Run the kernel; returns (out [1,S,E] float32, BassKernelResults)."""
    from concourse.bass_utils import run_bass_kernel_spmd

    nc = _get_nc()
    in_maps = _make_in_maps(
        inputs["x"], inputs["Wq"], inputs["bq"], inputs["Wk"], inputs["bk"],
        inputs["Wv"], inputs["bv"], inputs["Wo"],
    )
    res = run_bass_kernel_spmd(
        nc, in_maps, core_ids=list(range(N_CORES)), trace=trace
    )
    acc = np.zeros((SEQ, EMBED), np.float64)
    for c in range(N_CORES):
        acc += res.results[c]["out"]
    acc += np.asarray(inputs["bo"], np.float64)
    return acc.astype(np.float32).reshape(1, SEQ, EMBED), res


def kernel(x, Wq, bq, Wk, bk, Wv, bv, Wo, bo):
    out, _ = run(dict(x=x, Wq=Wq, bq=bq, Wk=Wk, bk=bk, Wv=Wv, bv=bv, Wo=Wo, bo=bo))
    return out


# revision 34
# speedup vs baseline: 1.6491x; 1.1308x over previous
"""TRN2 Bass/Tile kernel: 16-head MHA, B=1 S=4096 E=1024, head-sharded over 8 cores.

Sharding: tensor-parallel over heads. Core c owns heads {2c, 2c+1}: columns
[128c, 128(c+1)) of Wq/Wk/Wv (+bias slices) and rows [128c, 128(c+1)) of Wo.
Each core computes attention for its 2 heads and a partial out-projection
[S, E]; the host sums the 8 partials and adds bo (TP row-parallel unshard).

v2 design (PE-queue-bound baseline was 464us: MATMUL 336 + serial LDWEIGHTS 126):
  A) QT/KT/VT [128ch, S] = W_c^T @ x^T   (lhsT=W-slice, rhs=xT tiles, +bias on DVE)
     KT natural (no per-head zero-padding).
  B) V2 [128k, kt, 130] = [V_h0|ones|V_h1|ones] via PE transpose (l-sum ride-along)
  C) per 512-q block, per key-tile kt:
     - QK row-tiled: two concurrent K=64 matmuls (tile_position (0,0)/(64,0))
       -> scores^T [128k, 1024] = [h0 512q | h1 512q] in one PSUM pair
     - exp split by kt parity: even kt on ACT (true exp, scale=1/8), odd kt on
       DVE via Schraudolph fp16-bit trick: int16 = round(a*s + b) bitcast fp16
       (a = 1024*log2e/8, b = 15*1024 + C). Softmax renormalization absorbs
       the approximation's constant factor; mixed-tile error ~5e-3 rel (sim).
     - PV accumulate psum[65, 512] per head: rows 0:64 attn^T, row 64 = denom l
  D) per q-block, software-pipelined across the next blocks' kt loops:
     1/l via DVE reciprocal_approx_fast on the [1,512] l-row, partition-
     broadcast via a K=1 ones outer-product matmul (no DMA round trips),
     ATT = attn^T * (1/l), out-proj [q, E], evacuations split ACT/DVE.
"""

import sys

for _p in ("/opt/trn_rl_repo", "/opt/pypackages"):
    if _p not in sys.path:
        sys.path.append(_p)

import numpy as np

EMBED = 1024
N_CORES = 8
HC = EMBED // N_CORES  # 128 channels = 2 heads per core
DH = 64                # head dim
SEQ = 4096

# Schraudolph fp16 exp constants: exp(s/8) ~= bits_fp16(round(EXP_A*s + EXP_B))
EXP_A = 1024.0 * 1.4426950408889634 * 0.125
EXP_B = 15.0 * 1024.0 - 45.0

_NC_CACHE = {}


def _build_nc(S=SEQ, E=EMBED):
    from contextlib import ExitStack

    import concourse.bass as bass
    import concourse.mybir as mybir
    import concourse.tile as tile
    from concourse import bacc
    from concourse.masks import make_identity

    F32 = mybir.dt.float32
    MMDT = mybir.dt.float16
    I16 = mybir.dt.int16

    ET = E // 128      # 8 E-tiles of 128 (contraction for projections)
    NSC = S // 512     # 8 512-wide S chunks
    NKT = S // 128     # 32 128-wide key tiles
    NQS = 512 // 128   # 4 128-q subtiles per chunk
    NEC = E // 512     # 2 512-wide E chunks of the out-projection

    nc = bacc.Bacc()
    xT = nc.declare_dram_parameter("xT", [E, S], MMDT, isOutput=False)
    wq = nc.declare_dram_parameter("wq", [E, HC], MMDT, isOutput=False)
    wk = nc.declare_dram_parameter("wk", [E, HC], MMDT, isOutput=False)
    wv = nc.declare_dram_parameter("wv", [E, HC], MMDT, isOutput=False)
    bq = nc.declare_dram_parameter("bq", [HC, 1], F32, isOutput=False)
    bk = nc.declare_dram_parameter("bk", [HC, 1], F32, isOutput=False)
    bv = nc.declare_dram_parameter("bv", [HC, 1], F32, isOutput=False)
    wo = nc.declare_dram_parameter("wo", [HC, E], MMDT, isOutput=False)
    out = nc.declare_dram_parameter("out", [S, E], F32, isOutput=True)

    with tile.TileContext(nc) as tc, ExitStack() as ctx:
        wpool = ctx.enter_context(tc.tile_pool(name="w", bufs=1))
        xpool = ctx.enter_context(tc.tile_pool(name="x", bufs=8))
        qkvpool = ctx.enter_context(tc.tile_pool(name="qkv", bufs=1))
        v2pool = ctx.enter_context(tc.tile_pool(name="v2", bufs=1))
        epool = ctx.enter_context(tc.tile_pool(name="e", bufs=6))
        apool = ctx.enter_context(tc.tile_pool(name="a", bufs=3))
        rpool = ctx.enter_context(tc.tile_pool(name="r", bufs=4))
        # PSUM 8 banks: scores double-buffer 2x[128,1024]=4, PV 2x[65,512]=2,
        # transpose/out-proj 2x[128,512]=2
        spsum = ctx.enter_context(tc.tile_pool(name="sp", bufs=2, space="PSUM"))
        pvpsum = ctx.enter_context(tc.tile_pool(name="pv", bufs=2, space="PSUM"))
        opsum = ctx.enter_context(tc.tile_pool(name="op", bufs=2, space="PSUM"))

        # --- weights / constants ---
        w_sb = {}
        for name, src in (("wq", wq), ("wk", wk), ("wv", wv)):
            t = wpool.tile([128, ET, HC], MMDT, tag=name, name=name)
            nc.sync.dma_start(out=t, in_=src.rearrange("(a p) c -> p a c", p=128))
            w_sb[name] = t
        wo_sb = wpool.tile([HC, E], MMDT, tag="wo")
        nc.sync.dma_start(out=wo_sb, in_=wo[:, :])
        b_sb = {}
        for name, src in (("bq", bq), ("bk", bk), ("bv", bv)):
            t = wpool.tile([HC, 1], F32, tag=name, name=name)
            nc.sync.dma_start(out=t, in_=src[:, :])
            b_sb[name] = t
        ident = wpool.tile([128, 128], MMDT, tag="ident")
        make_identity(nc, ident)

        # --- stage A: QT/KT/VT [128ch, S] chunked by 512, natural head layout ---
        QT = [qkvpool.tile([HC, 512], MMDT, tag=f"qt{i}", name=f"qt{i}") for i in range(NSC)]
        KT = [qkvpool.tile([HC, 512], MMDT, tag=f"kt{i}", name=f"kt{i}") for i in range(NSC)]
        VT = [qkvpool.tile([HC, 512], MMDT, tag=f"vt{i}", name=f"vt{i}") for i in range(NSC)]
        for sc in range(NSC):
            big1 = spsum.tile([128, 1024], F32, tag="big")
            big2 = opsum.tile([128, 512], F32, tag="pt_po")
            for et in range(ET):
                xt = xpool.tile([128, 512], MMDT, tag="xt")
                nc.sync.dma_start(
                    out=xt, in_=xT[et * 128:(et + 1) * 128, sc * 512:(sc + 1) * 512]
                )
                first, last = et == 0, et == ET - 1
                nc.tensor.matmul(big1[:, 0:512], lhsT=w_sb["wq"][:, et, :],
                                 rhs=xt, start=first, stop=last)
                nc.tensor.matmul(big1[:, 512:1024], lhsT=w_sb["wk"][:, et, :],
                                 rhs=xt, start=first, stop=last)
                nc.tensor.matmul(big2[:, 0:512], lhsT=w_sb["wv"][:, et, :],
                                 rhs=xt, start=first, stop=last)
            nc.vector.tensor_scalar_add(QT[sc], big1[:, 0:512], b_sb["bq"])
            nc.vector.tensor_scalar_add(KT[sc], big1[:, 512:1024], b_sb["bk"])
            nc.vector.tensor_scalar_add(VT[sc], big2[:, 0:512], b_sb["bv"])

        # --- stage B: V2 [128k, NKT, 195] = [V_h0|ones|V_h1|ones|zero-pad] ---
        # (PV lhsT is padded to M=128 — full-width stationary loads measure
        # faster than M=65 — so each head's slice drags in 63 junk columns
        # whose PSUM rows are simply never read; the pad keeps them finite.)
        V2 = v2pool.tile([128, NKT, 195], MMDT, tag="V2")
        nc.vector.memset(V2[:, :, 64:65], 1.0)
        nc.vector.memset(V2[:, :, 129:130], 1.0)
        nc.vector.memset(V2[:, :, 130:195], 0.0)
        for kt in range(NKT):
            pt = opsum.tile([128, 512], MMDT, tag="pt_po")
            nc.tensor.transpose(
                pt[:, 0:128], VT[kt // 4][:, (kt % 4) * 128:(kt % 4 + 1) * 128], ident
            )
            # one copy: [128, 2, 64] -> V2 cols {0:64, 65:129}
            dst = bass.AP(
                tensor=V2.tensor, offset=V2.offset + kt * 195,
                ap=[list(V2.ap[0]), [65, 2], [1, 64]],
            )
            src = bass.AP(
                tensor=pt.tensor, offset=pt.offset,
                ap=[list(pt.ap[0]), [64, 2], [1, 64]],
            )
            nc.vector.tensor_copy(dst, src)

        # --- stage C+D: per 512-q block over all 32 key tiles, both heads.
        # Stage D is split into three phases, each deferred one q-block
        # behind, so neither the strict-FIFO ACT/DVE queues nor the PE queue
        # ever wait on the reciprocal's DRAM round-trips:
        #   phase1(qb): kt loop + PV-psum evacuation + l-row spread DMAs
        #   phase2(qb): reciprocal + spread-back DMAs     (at end of qb+1)
        #   phase3(qb): ATT scale + out-proj + store      (at end of qb+2)
        def phase1(qb, tail3):
            """kt loop, software-pipelined emission: QK(kt) | exp(kt-1) |
            PV(kt-2), with the previous blocks' phase2/phase3 work (tail3)
            spread through the early iterations as PE/ACT/DVE filler."""
            pv0 = pvpsum.tile([128, 512], F32, tag="pv", name="pv0")
            pv1 = pvpsum.tile([128, 512], F32, tag="pv", name="pv1")
            stiles = {}
            extiles = {}

            def emit_qk(kt):
                s = spsum.tile([128, 1024], F32, tag="big")
                ktile = KT[kt // 4][:, (kt % 4) * 128:(kt % 4 + 1) * 128]
                nc.tensor.matmul(
                    s[:, 0:512], lhsT=ktile[0:DH, :], rhs=QT[qb][0:DH, :],
                    start=True, stop=True,
                )
                nc.tensor.matmul(
                    s[:, 512:1024], lhsT=ktile[DH:HC, :], rhs=QT[qb][DH:HC, :],
                    start=True, stop=True,
                )
                stiles[kt] = s

            def emit_exp(kt):
                s = stiles.pop(kt)
                ex = epool.tile([128, 1024], MMDT, tag="ex")
                if kt % 2 == 0:
                    nc.scalar.activation(
                        ex, s, mybir.ActivationFunctionType.Exp, scale=0.125,
                    )
                else:
                    nc.vector.tensor_scalar(
                        out=ex.bitcast(I16), in0=s,
                        scalar1=EXP_A, scalar2=EXP_B,
                        op0=mybir.AluOpType.mult, op1=mybir.AluOpType.add,
                    )
                extiles[kt] = ex

            def emit_pv(kt):
                ex = extiles.pop(kt)
                first, last = kt == 0, kt == NKT - 1
                nc.tensor.matmul(
                    pv0, lhsT=V2[:, kt, 0:128], rhs=ex[:, 0:512],
                    start=first, stop=last,
                )
                nc.tensor.matmul(
                    pv1, lhsT=V2[:, kt, 65:193], rhs=ex[:, 512:1024],
                    start=first, stop=last,
                )

            toff = max(1, 12 - len(tail3)) if tail3 else 0
            for kt in range(NKT + 2):
                if kt < NKT:
                    emit_qk(kt)
                if tail3 and toff <= kt < toff + len(tail3):
                    tail3[kt - toff]()
                if kt >= 1 and kt - 1 < NKT:
                    emit_exp(kt - 1)
                if kt >= 2:
                    emit_pv(kt - 2)
            # Boundary evacuation: unnormalized attn^T (fp16) into one
            # [128, 512] tile (h0 rows 0:64 via ACT, h1 rows 64:128 via DVE)
            # and the two l-rows into a [2, 512] tile (DVE, partition-shift).
            pvcC = rpool.tile([128, 512], MMDT, tag="pvc", name="pvc")
            nc.scalar.copy(pvcC[0:DH, :], pv0[0:DH, :])
            nc.vector.tensor_copy(pvcC[DH:HC, :], pv1[0:DH, :])
            lrs = []
            for h, pv in ((0, pv0), (1, pv1)):
                lr = rpool.tile([1, 512], MMDT, tag=f"lr{h}", name="lr")
                nc.vector.tensor_copy(lr, pv[64:65, :])
                lrs.append(lr)
            return {"pvcC": pvcC, "lrs": lrs, "qb": qb}

        def phase2(st):
            """Transpose l to q-partitions via K=1 identity matmuls
            (lq[q, h*4+qs] columns), then one cheap [128, 8] reciprocal."""
            def lcols():
                lq = opsum.tile([128, 512], F32, tag="pt_po")
                st["lq"] = lq
                for h in range(2):
                    for qs in range(NQS):
                        nc.tensor.matmul(
                            lq[:, h * 4 + qs:h * 4 + qs + 1],
                            lhsT=st["lrs"][h][:, qs * 128:(qs + 1) * 128],
                            rhs=ident[0:1, 0:1],
                            start=True, stop=True,
                        )

            def recip():
                rl = rpool.tile([128, 8], F32, tag="rl", name="rl")
                nc.vector.reciprocal(rl, st["lq"][:, 0:8])
                st["rl"] = rl
            return [lcols, recip]

        def phase3(st, alt=False):
            """8x out-proj units: row-tiled per-head matmul pair (K=64,
            concurrent), then normalize-on-evacuation: ACT scales the h0
            partial by 1/l_h0 (per-partition scale), DVE fuses the h1
            partial's scale and the add."""
            qb = st["qb"]
            pvcC = st["pvcC"]

            def proj(qs, ec):
                def go():
                    rl = st["rl"]
                    po0 = opsum.tile([128, 512], F32, tag="pt_po")
                    po1 = opsum.tile([128, 512], F32, tag="pt_po")
                    qsl = slice(qs * 128, (qs + 1) * 128)
                    ecl = slice(ec * 512, (ec + 1) * 512)
                    nc.tensor.matmul(
                        po0, lhsT=pvcC[0:DH, qsl], rhs=wo_sb[0:DH, ecl],
                        start=True, stop=True,
                    )
                    nc.tensor.matmul(
                        po1, lhsT=pvcC[DH:HC, qsl], rhs=wo_sb[DH:HC, ecl],
                        start=True, stop=True,
                    )
                    osb = apool.tile([128, 512], F32, tag="osb")
                    nc.scalar.activation(
                        osb, po0, mybir.ActivationFunctionType.Copy,
                        scale=rl[:, qs:qs + 1],
                    )
                    nc.vector.scalar_tensor_tensor(
                        out=osb, in0=po1, scalar=rl[:, 4 + qs:5 + qs],
                        in1=osb, op0=mybir.AluOpType.mult,
                        op1=mybir.AluOpType.add,
                    )
                    nc.sync.dma_start(
                        out=out[qb * 512 + qs * 128:qb * 512 + (qs + 1) * 128,
                                ec * 512:(ec + 1) * 512],
                        in_=osb,
                    )
                return go

            return [proj(qs, ec) for qs in range(NQS) for ec in range(NEC)]

        pending = []
        ph2_prev = None
        for qb in range(NSC):
            tail3 = []
            if len(pending) >= 2:
                tail3 += phase3(pending.pop(0))
            if ph2_prev:
                tail3 += ph2_prev
            st = phase1(qb, tail3 or None)
            ph2_prev = phase2(st)
            pending.append(st)
        for fn in ph2_prev:
            fn()
        for fn in phase3(pending.pop(0), alt=True):
            fn()
        for fn in phase3(pending.pop(0), alt=True):
            fn()
    nc.finalize()
    return nc


def _get_nc(S=SEQ):
    key = S
    if key not in _NC_CACHE:
        _NC_CACHE[key] = _build_nc(S=S)
    return _NC_CACHE[key]


def _make_in_maps(x, Wq, bq, Wk, bk, Wv, bv, Wo, npdt=np.float16):
    xT = np.ascontiguousarray(np.asarray(x, np.float32)[0].T.astype(npdt))
    Wq, Wk, Wv, Wo = (np.asarray(a, np.float32).astype(npdt) for a in (Wq, Wk, Wv, Wo))
    bq, bk, bv = (np.asarray(a, np.float32) for a in (bq, bk, bv))
    in_maps = []
    for c in range(N_CORES):
        sl = slice(c * HC, (c + 1) * HC)
        in_maps.append({
            "xT": xT,
            "wq": np.ascontiguousarray(Wq[:, sl]),
            "wk": np.ascontiguousarray(Wk[:, sl]),
            "wv": np.ascontiguousarray(Wv[:, sl]),
            "bq": np.ascontiguousarray(bq[sl]).reshape(HC, 1),
            "bk": np.ascontiguousarray(bk[sl]).reshape(HC, 1),
            "bv": np.ascontiguousarray(bv[sl]).reshape(HC, 1),
            "wo": np.ascontiguousarray(Wo[sl, :]),
        })
    return in_maps


def run(inputs, trace=False, mmdt="fp16"):
    """Run the kernel; returns (out [1,S,E] float32, BassKernelResults)."""
    from concourse.bass_utils import run_bass_kernel_spmd

    nc = _get_nc()
    in_maps = _make_in_maps(
        inputs["x"], inputs["Wq"], inputs["bq"], inputs["Wk"], inputs["bk"],
        inputs["Wv"], inputs["bv"], inputs["Wo"],
    )
    res = run_bass_kernel_spmd(
        nc, in_maps, core_ids=list(range(N_CORES)), trace=trace
    )
    acc = np.zeros((SEQ, EMBED), np.float64)
    for c in range(N_CORES):
        acc += res.results[c]["out"]
    acc += np.asarray(inputs["bo"], np.float64)
    return acc.astype(np.float32).reshape(1, SEQ, EMBED), res


def kernel(x, Wq, bq, Wk, bk, Wv, bv, Wo, bo):
    out, _ = run(dict(x=x, Wq=Wq, bq=bq, Wk=Wk, bk=bk, Wv=Wv, bv=bv, Wo=Wo, bo=bo))
    return out
